# revision 1
# baseline (speedup 1.0000x reference)
"""Trainium2 Bass kernel for nn_DecoderTransformerBackbone_1589137900084.

Decoder transformer backbone: B=8, N=2048, D=256, L=4 layers of
relu-attention with a causal averaging mask + MLP, layernorms after each
residual. Data-parallel over batch: one batch element per NeuronCore (8 cores).

Per-core layout strategy:
  - H (the residual stream) lives in SBUF in normal layout as 16 tiles of
    [128 tokens, 256 dims].
  - Each layer PE-transposes H into HT [d, n] twice (for QKV and for the MLP),
    batching 4 [128,128] transposes per PSUM bank to amortize copy overhead.
  - qT/kT are produced transposed ([e, n]) with the weights stationary; v and
    the MLP2 output are produced in normal layout with activation tiles
    stationary.
  - S^T tiles ([j, i], 128x512) are computed only for the lower-triangular
    blocks; relu is fused into the PSUM->SBUF copy; the in-block triangle is
    applied by one [128,128] mask multiply on diagonal blocks; the 1/(i+1) row
    scale is fused into the attention residual via scalar_tensor_tensor.
  - All big matmuls run as float32r (single-pass fp32, ~1e-4 relative error);
    PE transposes run exact fp32.
"""
import sys

sys.path.insert(0, "/opt/trn_rl_repo")

import numpy as np

B, N, D, L = 8, 2048, 256, 4
LN_EPS = 1e-5
P = 128
NT = N // P            # 16 token tiles
DT = D // P            # 2 dim tiles
IC = N // 512          # 4 free-dim chunks of 512

_CACHE = {}
_last_in_maps = None
TRI_ON_GPSIMD = False
REPEAT = 1
PROFILE = False
LAST_EXEC_NS = None
LAST_RESULTS = None


def _build_program(use_b1, use_b2, use_ln1_gb, use_ln2_gb):
    import concourse.bass as bass  # noqa: F401
    from concourse import bacc
    import concourse.mybir as mybir
    import concourse.tile as tile

    f32 = mybir.dt.float32
    f32r = mybir.dt.float32r
    AF = mybir.ActivationFunctionType
    OP = mybir.AluOpType

    nc = bacc.Bacc("TRN2", target_bir_lowering=False)

    h0_d = nc.declare_dram_parameter("h0", [N, D], f32, isOutput=False)
    wq_d = nc.declare_dram_parameter("wq", [L, D, D], f32r, isOutput=False)
    wk_d = nc.declare_dram_parameter("wk", [L, D, D], f32r, isOutput=False)
    wv_d = nc.declare_dram_parameter("wv", [L, D, D], f32r, isOutput=False)
    w1_d = nc.declare_dram_parameter("w1", [L, D, D], f32r, isOutput=False)
    w2_d = nc.declare_dram_parameter("w2", [L, D, D], f32r, isOutput=False)
    tri_d = nc.declare_dram_parameter("tri", [P, P], f32, isOutput=False)
    ident_d = nc.declare_dram_parameter("ident", [P, P], f32, isOutput=False)
    invpos_d = nc.declare_dram_parameter("invpos", [P, NT], f32, isOutput=False)
    if use_b1:
        b1_d = nc.declare_dram_parameter("b1", [L, D], f32, isOutput=False)
    if use_b2:
        b2_d = nc.declare_dram_parameter("b2", [L, D], f32, isOutput=False)
    if use_ln1_gb:
        ln1g_d = nc.declare_dram_parameter("ln1g", [L, D], f32, isOutput=False)
        ln1b_d = nc.declare_dram_parameter("ln1b", [L, D], f32, isOutput=False)
    if use_ln2_gb:
        ln2g_d = nc.declare_dram_parameter("ln2g", [L, D], f32, isOutput=False)
        ln2b_d = nc.declare_dram_parameter("ln2b", [L, D], f32, isOutput=False)
    out_d = nc.declare_dram_parameter("out", [N, D], f32, isOutput=True)

    with tile.TileContext(nc) as tc:
        with (
            tc.tile_pool(name="const", bufs=1) as constp,
            tc.tile_pool(name="work", bufs=1) as workp,
            tc.tile_pool(name="stp", bufs=16) as stp,
            tc.tile_pool(name="sqp", bufs=3) as sqp,
            tc.tile_pool(name="small", bufs=8) as smallp,
            tc.tile_pool(name="pbig", bufs=2, space="PSUM") as pbig,
            tc.tile_pool(name="ps512", bufs=2, space="PSUM") as ps512,
            tc.tile_pool(name="pav", bufs=2, space="PSUM") as pavp,
        ):
            # ---------------- constants & weights ----------------
            tri = constp.tile([P, P], f32, tag="tri")
            ident = constp.tile([P, P], f32, tag="ident")
            invpos = constp.tile([P, NT], f32, tag="invpos")
            eps_t = constp.tile([P, 1], f32, tag="eps")
            nc.sync.dma_start(tri[:], tri_d[:])
            nc.sync.dma_start(ident[:], ident_d[:])
            nc.sync.dma_start(invpos[:], invpos_d[:])
            nc.vector.memset(eps_t[:], LN_EPS)

            W = {}
            for wname, wd in (("wq", wq_d), ("wk", wk_d), ("wv", wv_d),
                              ("w1", w1_d), ("w2", w2_d)):
                for l in range(L):
                    for dt_ in range(DT):
                        t = constp.tile([P, D], f32r, tag=f"{wname}_{l}_{dt_}")
                        nc.sync.dma_start(
                            t[:], wd[l, dt_ * P:(dt_ + 1) * P, :])
                        W[wname, l, dt_] = t

            def load_vec(dram, tag):
                out = []
                for l in range(L):
                    t = constp.tile([P, DT], f32, tag=f"{tag}_{l}")
                    nc.sync.dma_start(
                        t[:], dram[l].rearrange("(dt p) -> p dt", p=P))
                    out.append(t)
                return out

            def load_bcast(dram, tag):
                out = []
                for l in range(L):
                    t = constp.tile([P, D], f32, tag=f"{tag}b_{l}")
                    nc.sync.dma_start(
                        t[:], dram[l].unsqueeze(0).to_broadcast([P, D]))
                    out.append(t)
                return out

            b1_t = load_vec(b1_d, "b1") if use_b1 else None
            b2_t = load_bcast(b2_d, "b2") if use_b2 else None
            ln1g_t = load_bcast(ln1g_d, "ln1g") if use_ln1_gb else None
            ln1b_t = load_bcast(ln1b_d, "ln1b") if use_ln1_gb else None
            ln2g_t = load_bcast(ln2g_d, "ln2g") if use_ln2_gb else None
            ln2b_t = load_bcast(ln2b_d, "ln2b") if use_ln2_gb else None

            # ---------------- activations ----------------
            H = []
            for nt in range(NT):
                t = workp.tile([P, D], f32, tag=f"h_{nt}")
                nc.sync.dma_start(t[:], h0_d[nt * P:(nt + 1) * P, :])
                H.append(t)
            HT = [workp.tile([P, N], f32r, tag=f"ht_{d}", name=f"ht_{d}") for d in range(DT)]
            qT = [workp.tile([P, N], f32r, tag=f"qt_{d}", name=f"qt_{d}") for d in range(DT)]
            kT = [workp.tile([P, N], f32r, tag=f"kt_{d}", name=f"kt_{d}") for d in range(DT)]
            h1T = [workp.tile([P, N], f32r, tag=f"h1t_{d}", name=f"h1t_{d}") for d in range(DT)]
            Vp = [workp.tile([P, 2 * D], f32r, tag=f"vp_{i}", name=f"vp_{i}")
                  for i in range(NT // 2)]

            def Vsl(nt):
                return Vp[nt // 2][:, (nt % 2) * D:(nt % 2 + 1) * D]

            def psum_copy(dst, src, act):
                if act:
                    nc.scalar.activation(dst, src, AF.Copy)
                else:
                    nc.vector.tensor_copy(dst, src)

            def transpose_H_to_HT(flip):
                # 8 transposes per 2-bank psum tile, then one wide copy.
                for dt_ in range(DT):
                    for g in range(2):
                        ps = pbig.tile([P, 1024], f32, tag="pbig", name="ps_t")
                        for k in range(8):
                            nt = g * 8 + k
                            nc.tensor.transpose(
                                ps[:, k * P:(k + 1) * P],
                                H[nt][:, dt_ * P:(dt_ + 1) * P],
                                ident[:],
                            )
                        psum_copy(HT[dt_][:, g * 1024:(g + 1) * 1024], ps[:],
                                  act=((dt_ * 2 + g + flip) % 2 == 0))

            def ln_sumsq(h, dst, act):
                # sum(h^2) along free dim -> dst [P,1]; full pass over h.
                if act:
                    sq = sqp.tile([P, D], f32, tag="sqs", name="sqs")
                    nc.scalar.activation(sq[:], h[:], AF.Square, accum_out=dst)
                else:
                    sq = sqp.tile([P, D], f32, tag="sqv", name="sqv")
                    nc.vector.scalar_tensor_tensor(
                        out=sq[:], in0=h[:], scalar=1.0, in1=h[:],
                        op0=OP.mult, op1=OP.mult, accum_out=dst)

            def ln_group_stats(sum_g, ssq_g, n):
                # rstd = 1/sqrt((sumsq - sum^2/D)/D + eps); nmu = -sum/D*rstd
                sq = smallp.tile([P, 4], f32, tag="lnsq", name="lnsq")
                rstd = smallp.tile([P, 4], f32, tag="lnrstd", name="lnrstd")
                nmu = smallp.tile([P, 4], f32, tag="lnnmu", name="lnnmu")
                nc.vector.tensor_tensor(out=sq[:, :n], in0=sum_g[:, :n],
                                        in1=sum_g[:, :n], op=OP.mult)
                nc.vector.scalar_tensor_tensor(
                    out=sq[:, :n], in0=sq[:, :n], scalar=-1.0 / D,
                    in1=ssq_g[:, :n], op0=OP.mult, op1=OP.add)
                nc.scalar.activation(rstd[:, :n], sq[:, :n], AF.Sqrt,
                                     bias=eps_t[:], scale=1.0 / D)
                nc.vector.reciprocal(rstd[:, :n], rstd[:, :n])
                nc.vector.scalar_tensor_tensor(
                    out=nmu[:, :n], in0=sum_g[:, :n], scalar=-1.0 / D,
                    in1=rstd[:, :n], op0=OP.mult, op1=OP.mult)
                return rstd, nmu

            def ln_apply(h, rstd, nmu, k, act, g_t, b_t):
                # h = h*rstd + (-mu*rstd), then optional *g + b
                if act:
                    nc.scalar.activation(h[:], h[:], AF.Identity,
                                         scale=rstd[:, k:k + 1],
                                         bias=nmu[:, k:k + 1])
                else:
                    nc.vector.tensor_scalar(
                        out=h[:], in0=h[:], scalar1=rstd[:, k:k + 1],
                        scalar2=nmu[:, k:k + 1], op0=OP.mult, op1=OP.add)
                if g_t is not None:
                    nc.vector.tensor_tensor(out=h[:], in0=h[:], in1=g_t[:],
                                            op=OP.mult)
                    nc.vector.tensor_tensor(out=h[:], in0=h[:], in1=b_t[:],
                                            op=OP.add)

            for li in range(L * REPEAT):
                l = li % L
                # ---------- phase A: HT + QKV ----------
                transpose_H_to_HT(flip=0)
                for wi, (name, dst) in enumerate((("wq", qT), ("wk", kT))):
                    for et in range(DT):
                        for cp in range(2):
                            ps = pbig.tile([P, 1024], f32, tag="pbig",
                                           name="ps_qk")
                            for half in range(2):
                                ic = cp * 2 + half
                                for dt_ in range(DT):
                                    nc.tensor.matmul(
                                        ps[:, half * 512:(half + 1) * 512],
                                        W[name, l, dt_][:, et * P:(et + 1) * P],
                                        HT[dt_][:, ic * 512:(ic + 1) * 512],
                                        start=(dt_ == 0), stop=(dt_ == DT - 1),
                                        skip_group_check=True,
                                    )
                            psum_copy(dst[et][:, cp * 1024:(cp + 1) * 1024],
                                      ps[:], act=((wi + et + cp) % 2 == 0))
                for pair in range(NT // 2):
                    ps = pbig.tile([P, 1024], f32, tag="pbig", name="ps_v")
                    for k in range(2):
                        nt = 2 * pair + k
                        for dt_ in range(DT):
                            nc.tensor.matmul(
                                ps[:, k * 512:k * 512 + D],
                                HT[dt_][:, nt * P:(nt + 1) * P],
                                W["wv", l, dt_][:],
                                start=(dt_ == 0), stop=(dt_ == DT - 1),
                                skip_group_check=True,
                            )
                    psum_copy(
                        Vp[pair].rearrange("p (b x) -> p b x", b=2),
                        ps.rearrange("p (b x) -> p b x", b=2)[:, :, :D],
                        act=(pair % 2 == 0))

                # ---------- phase B: attention ----------
                for ic in range(IC):
                    STl = []
                    for jt in range(4 * ic + 4):
                        c0 = P * max(0, jt - 4 * ic)
                        # keep produced width >= 256 so fp32r runs 1 cyc/row
                        c0p = min(c0, 512 - 256)
                        ps = ps512.tile([P, 512], f32, tag="ps512",
                                        name="ps_s")
                        for et in range(DT):
                            nc.tensor.matmul(
                                ps[:, c0p:],
                                kT[et][:, jt * P:(jt + 1) * P],
                                qT[et][:, ic * 512 + c0p:(ic + 1) * 512],
                                start=(et == 0), stop=(et == DT - 1),
                            )
                        st = stp.tile([P, 512], f32r, tag="st", name="st")
                        if jt % 2 == 0:
                            nc.scalar.activation(st[:, c0:], ps[:, c0:],
                                                 AF.Relu)
                        else:
                            nc.vector.tensor_scalar_max(st[:, c0:],
                                                        ps[:, c0:], 0.0)
                        if jt >= 4 * ic:
                            eng = nc.gpsimd if TRI_ON_GPSIMD else nc.vector
                            eng.tensor_tensor(
                                out=st[:, c0:c0 + P], in0=st[:, c0:c0 + P],
                                in1=tri[:], op=OP.mult)
                        STl.append(st)
                    sum_g = smallp.tile([P, 4], f32, tag="sumg", name="sumg")
                    ssq_g = smallp.tile([P, 4], f32, tag="ssqg", name="ssqg")
                    for ib_l in range(4):
                        ib = 4 * ic + ib_l
                        av = pavp.tile([P, D], f32, tag="pav", name="pav")
                        for jt in range(ib + 1):
                            nc.tensor.matmul(
                                av[:],
                                STl[jt][:, ib_l * P:(ib_l + 1) * P],
                                Vsl(jt),
                                start=(jt == 0), stop=(jt == ib),
                            )
                        nc.vector.scalar_tensor_tensor(
                            out=H[ib][:], in0=av[:],
                            scalar=invpos[:, ib:ib + 1], in1=H[ib][:],
                            op0=OP.mult, op1=OP.add,
                            accum_out=sum_g[:, ib_l:ib_l + 1])
                        ln_sumsq(H[ib], ssq_g[:, ib_l:ib_l + 1],
                                 act=(ib_l % 2 == 0))
                    rstd, nmu = ln_group_stats(sum_g, ssq_g, 4)
                    for ib_l in range(4):
                        ib = 4 * ic + ib_l
                        ln_apply(H[ib], rstd, nmu, ib_l, act=(ib_l % 2 == 1),
                                 g_t=ln1g_t[l] if use_ln1_gb else None,
                                 b_t=ln1b_t[l] if use_ln1_gb else None)

                # ---------- phase D: MLP ----------
                transpose_H_to_HT(flip=1)
                for et in range(DT):
                    for cp in range(2):
                        ps = pbig.tile([P, 1024], f32, tag="pbig",
                                       name="ps_h1")
                        for half in range(2):
                            ic = cp * 2 + half
                            for dt_ in range(DT):
                                nc.tensor.matmul(
                                    ps[:, half * 512:(half + 1) * 512],
                                    W["w1", l, dt_][:, et * P:(et + 1) * P],
                                    HT[dt_][:, ic * 512:(ic + 1) * 512],
                                    start=(dt_ == 0), stop=(dt_ == DT - 1),
                                    skip_group_check=True,
                                )
                        if use_b1:
                            nc.scalar.activation(
                                h1T[et][:, cp * 1024:(cp + 1) * 1024], ps[:],
                                AF.Relu, bias=b1_t[l][:, et:et + 1])
                        else:
                            nc.scalar.activation(
                                h1T[et][:, cp * 1024:(cp + 1) * 1024], ps[:],
                                AF.Relu)
                for g in range(NT // 4):
                    sum_g = smallp.tile([P, 4], f32, tag="sumg", name="sumg2")
                    ssq_g = smallp.tile([P, 4], f32, tag="ssqg", name="ssqg2")
                    for k in range(4):
                        nt = 4 * g + k
                        ps = pavp.tile([P, D], f32, tag="pav", name="ps_m")
                        for et in range(DT):
                            nc.tensor.matmul(
                                ps[:],
                                h1T[et][:, nt * P:(nt + 1) * P],
                                W["w2", l, et][:],
                                start=(et == 0), stop=(et == DT - 1),
                            )
                        if use_b2:
                            nc.vector.scalar_tensor_tensor(
                                out=H[nt][:], in0=ps[:], scalar=1.0,
                                in1=H[nt][:], op0=OP.mult, op1=OP.add)
                            nc.vector.tensor_tensor(out=H[nt][:], in0=H[nt][:],
                                                    in1=b2_t[l][:], op=OP.add)
                            nc.vector.tensor_reduce(
                                out=sum_g[:, k:k + 1], in_=H[nt][:],
                                axis=mybir.AxisListType.X, op=OP.add)
                        else:
                            nc.vector.scalar_tensor_tensor(
                                out=H[nt][:], in0=ps[:], scalar=1.0,
                                in1=H[nt][:], op0=OP.mult, op1=OP.add,
                                accum_out=sum_g[:, k:k + 1])
                        ln_sumsq(H[nt], ssq_g[:, k:k + 1], act=(k % 2 == 0))
                    rstd, nmu = ln_group_stats(sum_g, ssq_g, 4)
                    for k in range(4):
                        nt = 4 * g + k
                        ln_apply(H[nt], rstd, nmu, k, act=(k % 2 == 1),
                                 g_t=ln2g_t[l] if use_ln2_gb else None,
                                 b_t=ln2b_t[l] if use_ln2_gb else None)

                # write back after last layer
                if li == L * REPEAT - 1:
                    for nt in range(NT):
                        nc.sync.dma_start(
                            out_d[nt * P:(nt + 1) * P, :], H[nt][:])

    nc.finalize()
    return nc


def kernel(**inputs):
    global LAST_EXEC_NS, LAST_RESULTS
    from concourse import bass_utils

    x = np.asarray(inputs["x"], dtype=np.float32)
    wpe = np.asarray(inputs["wpe"], dtype=np.float32)
    assert x.shape == (B, N, D), x.shape

    use_b1 = bool(np.any(np.asarray(inputs["mlp_b1"]) != 0))
    use_b2 = bool(np.any(np.asarray(inputs["mlp_b2"]) != 0))
    use_ln1 = not (np.all(np.asarray(inputs["ln1_g"]) == 1)
                   and np.all(np.asarray(inputs["ln1_b"]) == 0))
    use_ln2 = not (np.all(np.asarray(inputs["ln2_g"]) == 1)
                   and np.all(np.asarray(inputs["ln2_b"]) == 0))

    key = (use_b1, use_b2, use_ln1, use_ln2)
    if key not in _CACHE:
        _CACHE[key] = _build_program(*key)
    nc = _CACHE[key]

    h0 = x + wpe[None, :, :]  # positional embedding folded in on host

    tri = np.tril(np.ones((P, P), dtype=np.float32)).T  # tri[j,i] = j<=i
    ident = np.eye(P, dtype=np.float32)
    pos = np.arange(N, dtype=np.float32).reshape(NT, P).T  # [P, NT]
    invpos = (1.0 / (pos + 1.0)).astype(np.float32)

    shared = {
        "wq": np.ascontiguousarray(inputs["Wq"], dtype=np.float32),
        "wk": np.ascontiguousarray(inputs["Wk"], dtype=np.float32),
        "wv": np.ascontiguousarray(inputs["Wv"], dtype=np.float32),
        "w1": np.ascontiguousarray(inputs["mlp_W1"], dtype=np.float32),
        "w2": np.ascontiguousarray(inputs["mlp_W2"], dtype=np.float32),
        "tri": tri, "ident": ident, "invpos": invpos,
    }
    if use_b1:
        shared["b1"] = np.asarray(inputs["mlp_b1"], dtype=np.float32)
    if use_b2:
        shared["b2"] = np.asarray(inputs["mlp_b2"], dtype=np.float32)
    if use_ln1:
        shared["ln1g"] = np.asarray(inputs["ln1_g"], dtype=np.float32)
        shared["ln1b"] = np.asarray(inputs["ln1_b"], dtype=np.float32)
    if use_ln2:
        shared["ln2g"] = np.asarray(inputs["ln2_g"], dtype=np.float32)
        shared["ln2b"] = np.asarray(inputs["ln2_b"], dtype=np.float32)

    in_maps = [dict(shared, h0=np.ascontiguousarray(h0[c])) for c in range(B)]
    global _last_in_maps
    _last_in_maps = in_maps

    res = bass_utils.run_bass_kernel_spmd(
        nc, in_maps, core_ids=list(range(B)), trace=PROFILE)
    LAST_EXEC_NS = res.exec_time_ns
    LAST_RESULTS = res
    return np.stack([res.results[c]["out"] for c in range(B)], axis=0)



# revision 40
# speedup vs baseline: 1.3599x; 1.3599x over previous
"""Trainium2 Bass kernel for nn_DecoderTransformerBackbone_1589137900084.

Decoder transformer backbone: B=8, N=2048, D=256, L=4 layers of
relu-attention with a causal averaging mask + MLP, layernorms after each
residual. Data-parallel over batch: one batch element per NeuronCore (8 cores).

Design (v3):
  - All matmul operands are bf16 (1 cyc/row on PE incl. transposes); the
    residual stream H is bf16; PSUM accumulation is fp32. Numpy emulation of
    this quantization gives rel_err ~1.1e-2 < 2e-2.
  - Wk is folded into Wq on the host: M = Wq @ Wk^T, so scores are
    S = (H M) H^T and the k projection disappears.
  - The causal averaging scale 1/(i+1) is folded into the q~T PSUM->SBUF
    copy (elementwise multiply with a broadcast inverse-position row), so
    the attention residual update is a plain fused add.
  - Elementwise split: Act takes wide PSUM relu/copies (V, full score tiles,
    h1), DVE takes HT copies, q~ scale, diagonal score tiles, residual adds
    (fused row-sum) and sum-of-squares (4x bf16 mode), Pool (gpsimd) takes
    the SBUF-only LN applies (except tail chunks, which go to DVE to
    shorten the critical chain).
  - Full software pipelining: HT is double-buffered (attn vs MLP), MLP
    transposes interleave into the attention phase, and the next layer's
    phase A (transpose + q~ + V) interleaves into this layer's MLP phase,
    so PE never waits for an LN tail.
"""
import sys

sys.path.insert(0, "/opt/trn_rl_repo")

import numpy as np

B, N, D, L = 8, 2048, 256, 4
LN_EPS = 1e-5
P = 128
NT = N // P            # 16 token tiles
DT = D // P            # 2 dim tiles
CH = 4                 # chunks of 512 tokens
CW = N // CH           # 512

_CACHE = {}
_last_in_maps = None
REPEAT = 1
PROFILE = False
LAST_EXEC_NS = None
LAST_RESULTS = None


def _build_program(use_b1, use_b2, use_ln1_gb, use_ln2_gb):
    import concourse.bass as bass  # noqa: F401
    from concourse import bacc
    import concourse.mybir as mybir
    import concourse.tile as tile

    f32 = mybir.dt.float32
    bf16 = mybir.dt.bfloat16
    AF = mybir.ActivationFunctionType
    OP = mybir.AluOpType

    nc = bacc.Bacc("TRN2", target_bir_lowering=False)

    ident_d = nc.declare_dram_parameter("ident", [P, P], bf16, isOutput=False)
    tri_d = nc.declare_dram_parameter("tri", [4, P, CW], bf16, isOutput=False)
    invpos_d = nc.declare_dram_parameter("invpos", [P, NT], f32,
                                         isOutput=False)
    h0_d = nc.declare_dram_parameter("h0", [N, D], bf16, isOutput=False)
    h0t_d = nc.declare_dram_parameter("h0t", [D, N], bf16, isOutput=False)
    m_d = nc.declare_dram_parameter("m", [L, D, D], bf16, isOutput=False)
    wv_d = nc.declare_dram_parameter("wv", [L, D, D], bf16, isOutput=False)
    w1_d = nc.declare_dram_parameter("w1", [L, D, D], bf16, isOutput=False)
    w2_d = nc.declare_dram_parameter("w2", [L, D, D], bf16, isOutput=False)
    if use_b1:
        b1_d = nc.declare_dram_parameter("b1", [L, D], f32, isOutput=False)
    if use_b2:
        b2_d = nc.declare_dram_parameter("b2", [L, D], bf16, isOutput=False)
    if use_ln1_gb:
        ln1g_d = nc.declare_dram_parameter("ln1g", [L, D], bf16, isOutput=False)
        ln1b_d = nc.declare_dram_parameter("ln1b", [L, D], bf16, isOutput=False)
    if use_ln2_gb:
        ln2g_d = nc.declare_dram_parameter("ln2g", [L, D], bf16, isOutput=False)
        ln2b_d = nc.declare_dram_parameter("ln2b", [L, D], bf16, isOutput=False)
    out_d = nc.declare_dram_parameter("out", [N, D], bf16, isOutput=True)

    with tile.TileContext(nc) as tc:
        with (
            tc.tile_pool(name="const", bufs=1) as constp,
            tc.tile_pool(name="work", bufs=1) as workp,
            tc.tile_pool(name="stp", bufs=40) as stp,
            tc.tile_pool(name="sqp", bufs=3) as sqp,
            tc.tile_pool(name="small", bufs=24) as smallp,
            tc.tile_pool(name="ptr", bufs=1, space="PSUM") as ptrans,
            tc.tile_pool(name="pq", bufs=2, space="PSUM") as pqp,
            tc.tile_pool(name="psc", bufs=2, space="PSUM") as pscore,
            tc.tile_pool(name="pav", bufs=3, space="PSUM") as pavp,
        ):
            # ---------------- constants, inputs, weights ----------------
            # DMA order matters for startup: SP queue carries the layer-0
            # critical path (transposed h0, invpos, first H tiles, masks);
            # the Act queue carries weights in layer order.
            HTa = [workp.tile([P, N], bf16, tag=f"hta_{d}", name=f"hta_{d}")
                   for d in range(DT)]
            invpos = constp.tile([P, NT], f32, tag="invpos")
            tri = constp.tile([P, 4 * CW], bf16, tag="tri")
            HN = N // 2
            # first halves of h0^T + invpos + masks: layer-0 critical path
            for dt_ in range(DT):
                nc.sync.dma_start(HTa[dt_][:, :HN],
                                  h0t_d[dt_ * P:(dt_ + 1) * P, :HN])
            nc.sync.dma_start(invpos[:], invpos_d[:])
            for k in range(4):
                nc.sync.dma_start(tri[:, k * CW:(k + 1) * CW], tri_d[k])
            for dt_ in range(DT):
                nc.sync.dma_start(HTa[dt_][:, HN:],
                                  h0t_d[dt_ * P:(dt_ + 1) * P, HN:])
            H = []
            for nt in range(NT):
                t = workp.tile([P, D], bf16, tag=f"h_{nt}")
                nc.sync.dma_start(t[:], h0_d[nt * P:(nt + 1) * P, :])
                H.append(t)

            W = {}

            def load_weights(l, eng):
                for wname, wd in (("m", m_d), ("wv", wv_d), ("w1", w1_d),
                                  ("w2", w2_d)):
                    for dt_ in range(DT):
                        t = constp.tile([P, D], bf16, tag=f"{wname}_{l}_{dt_}")
                        eng.dma_start(
                            t[:], wd[l, dt_ * P:(dt_ + 1) * P, :])
                        W[wname, l, dt_] = t

            ident = constp.tile([P, P], bf16, tag="ident")
            eps_t = constp.tile([P, 1], f32, tag="eps")
            # m/wv for layer 0 race the prologue on the otherwise-idle Act
            # queue; everything else goes behind the h0 loads on SP.
            for wname, wd in (("m", m_d), ("wv", wv_d)):
                for dt_ in range(DT):
                    t = constp.tile([P, D], bf16, tag=f"{wname}_0_{dt_}")
                    nc.scalar.dma_start(t[:], wd[0, dt_ * P:(dt_ + 1) * P, :])
                    W[wname, 0, dt_] = t
            for wname, wd in (("w1", w1_d), ("w2", w2_d)):
                for dt_ in range(DT):
                    t = constp.tile([P, D], bf16, tag=f"{wname}_0_{dt_}")
                    nc.sync.dma_start(t[:], wd[0, dt_ * P:(dt_ + 1) * P, :])
                    W[wname, 0, dt_] = t
            nc.sync.dma_start(ident[:], ident_d[:])
            for l in range(1, L):
                load_weights(l, nc.sync)
            nc.vector.memset(eps_t[:], LN_EPS)

            def trimask(k):
                return tri[:, k * CW:(k + 1) * CW]

            def load_vec_f32(dram, tag):
                out = []
                for l in range(L):
                    t = constp.tile([P, DT], f32, tag=f"{tag}_{l}")
                    nc.sync.dma_start(
                        t[:], dram[l].rearrange("(dt p) -> p dt", p=P))
                    out.append(t)
                return out

            def load_bcast(dram, tag):
                out = []
                for l in range(L):
                    t = constp.tile([P, D], bf16, tag=f"{tag}b_{l}")
                    nc.sync.dma_start(
                        t[:], dram[l].unsqueeze(0).to_broadcast([P, D]))
                    out.append(t)
                return out

            b1_t = load_vec_f32(b1_d, "b1") if use_b1 else None
            b2_t = load_bcast(b2_d, "b2") if use_b2 else None
            ln1g_t = load_bcast(ln1g_d, "ln1g") if use_ln1_gb else None
            ln1b_t = load_bcast(ln1b_d, "ln1b") if use_ln1_gb else None
            ln2g_t = load_bcast(ln2g_d, "ln2g") if use_ln2_gb else None
            ln2b_t = load_bcast(ln2b_d, "ln2b") if use_ln2_gb else None

            # HTm: MLP-phase transpose of H (HTa declared above, DMA-seeded)
            HTm = [workp.tile([P, N], bf16, tag=f"htm_{d}", name=f"htm_{d}")
                   for d in range(DT)]
            qT = [workp.tile([P, N], bf16, tag=f"qt_{d}", name=f"qt_{d}")
                  for d in range(DT)]
            h1T = [workp.tile([P, N], bf16, tag=f"h1t_{d}", name=f"h1t_{d}")
                   for d in range(DT)]
            Vp = [workp.tile([P, 2 * D], bf16, tag=f"vp_{i}", name=f"vp_{i}")
                  for i in range(NT // 2)]

            def Vsl(nt):
                return Vp[nt // 2][:, (nt % 2) * D:(nt % 2 + 1) * D]

            # ---------------- helpers ----------------
            def transpose_chunk(HT, c):
                """PE-transpose H tiles of chunk c into HT[dt][:, c*512:...]."""
                ps = ptrans.tile([P, 2 * CW], bf16, tag="ptr", name="ps_tr")
                for dt_ in range(DT):
                    for k in range(4):
                        nc.tensor.transpose(
                            ps[:, dt_ * CW + k * P: dt_ * CW + (k + 1) * P],
                            H[4 * c + k][:, dt_ * P:(dt_ + 1) * P],
                            ident[:],
                        )
                for dt_ in range(DT):
                    nc.vector.tensor_copy(
                        HT[dt_][:, c * CW:(c + 1) * CW],
                        ps[:, dt_ * CW:(dt_ + 1) * CW])

            def qproj_chunk(l, c):
                """q~T chunk: qT[et][:, cW:] = M^T H^T (plain Act copy)."""
                for et in range(DT):
                    ps = pqp.tile([P, CW], f32, tag="pq", name="ps_q")
                    for dt_ in range(DT):
                        nc.tensor.matmul(
                            ps[:],
                            W["m", l, dt_][:, et * P:(et + 1) * P],
                            HTa[dt_][:, c * CW:(c + 1) * CW],
                            start=(dt_ == 0), stop=(dt_ == DT - 1),
                            skip_group_check=True,
                        )
                    nc.scalar.activation(
                        qT[et][:, c * CW:(c + 1) * CW], ps[:], AF.Copy)

            def vproj_chunk(l, c):
                """V for the two tile-pairs of chunk c (normal layout)."""
                for pr in range(2):
                    pair = 2 * c + pr
                    ps = pqp.tile([P, CW], f32, tag="pq", name="ps_v")
                    for k in range(2):
                        nt = 2 * pair + k
                        for dt_ in range(DT):
                            nc.tensor.matmul(
                                ps[:, k * D:(k + 1) * D],
                                HTa[dt_][:, nt * P:(nt + 1) * P],
                                W["wv", l, dt_][:],
                                start=(dt_ == 0), stop=(dt_ == DT - 1),
                                skip_group_check=True,
                            )
                    nc.scalar.activation(Vp[pair][:], ps[:], AF.Copy)

            ST = {}

            relu_rr = [0]

            def scores_chunk(c):
                """S~^T tiles for chunk c: jt in [0, 4c+3]. Score PSUM
                alternates between psc and the phase-B-idle pq pool (4
                effective bufs); full-tile relus rotate Act/Act/DVE."""
                for jt in range(4 * c + 4):
                    k = jt - 4 * c
                    c0 = P * max(0, k)
                    pool, ptag = ((pscore, "psc") if jt % 2 == 0
                                  else (pqp, "pq"))
                    ps = pool.tile([P, CW], f32, tag=ptag, name="ps_s")
                    for et in range(DT):
                        nc.tensor.matmul(
                            ps[:, c0:],
                            HTa[et][:, jt * P:(jt + 1) * P],
                            qT[et][:, c * CW + c0:(c + 1) * CW],
                            start=(et == 0), stop=(et == DT - 1),
                        )
                    st = stp.tile([P, CW], bf16, tag="st", name="st")
                    if k >= 0:
                        # diagonal tile: relu + in-tile triangle mask in one
                        # DVE op (stt max,mult -- HW-validated)
                        nc.vector.scalar_tensor_tensor(
                            out=st[:, c0:], in0=ps[:, c0:], scalar=0.0,
                            in1=trimask(k)[:, c0:], op0=OP.max, op1=OP.mult)
                    elif relu_rr[0] % 4 == 3:
                        nc.vector.tensor_scalar_max(st[:], ps[:], 0.0)
                        relu_rr[0] += 1
                    else:
                        nc.scalar.activation(st[:], ps[:], AF.Relu)
                        relu_rr[0] += 1
                    ST[c, jt] = st

            def ln_group_stats(sum_g, ssq_g, n):
                # std = sqrt((ssq - D*mu^2)/D + eps); rstd/nmu are computed
                # lazily in the deferred apply (after the Act sqrt) so no
                # DVE-queue op ever waits cross-engine.
                mun = smallp.tile([P, 4], f32, tag="lnmu", name="lnmu")
                t1 = smallp.tile([P, 4], f32, tag="lnt1", name="lnt1")
                std = smallp.tile([P, 4], f32, tag="lnstd", name="lnstd")
                nc.vector.tensor_scalar(
                    out=mun[:, :n], in0=sum_g[:, :n], scalar1=-1.0 / D,
                    scalar2=0.0, op0=OP.mult, op1=OP.add)
                nc.vector.tensor_tensor(out=t1[:, :n], in0=mun[:, :n],
                                        in1=mun[:, :n], op=OP.mult)
                nc.vector.scalar_tensor_tensor(
                    out=t1[:, :n], in0=t1[:, :n], scalar=-float(D),
                    in1=ssq_g[:, :n], op0=OP.mult, op1=OP.add)
                nc.scalar.activation(std[:, :n], t1[:, :n], AF.Sqrt,
                                     bias=eps_t[:], scale=1.0 / D)
                return mun, std

            def ln_finish(mun, std, n=4):
                # rstd = 1/std (DVE); nmu = mun * rstd (DVE, in-order)
                rstd = smallp.tile([P, 4], f32, tag="lnrstd", name="lnrstd")
                nmu = smallp.tile([P, 4], f32, tag="lnnmu", name="lnnmu")
                nc.vector.reciprocal(rstd[:, :n], std[:, :n])
                nc.vector.scalar_tensor_tensor(
                    out=nmu[:, :n], in0=mun[:, :n], scalar=1.0,
                    in1=rstd[:, :n], op0=OP.mult, op1=OP.mult)
                return rstd, nmu

            def resid_stats(nt, ps, sum_g, ssq_g, kk, scale=None):
                """H[nt] += scale*ps (PSUM); fused row-sum + sumsq on DVE."""
                nc.vector.scalar_tensor_tensor(
                    out=H[nt][:], in0=ps[:],
                    scalar=(1.0 if scale is None else scale), in1=H[nt][:],
                    op0=OP.mult, op1=OP.add,
                    accum_out=sum_g[:, kk:kk + 1])
                sq = sqp.tile([P, D], bf16, tag="sq", name="sq")
                nc.vector.scalar_tensor_tensor(
                    out=sq[:], in0=H[nt][:], scalar=1.0, in1=H[nt][:],
                    op0=OP.mult, op1=OP.mult,
                    accum_out=ssq_g[:, kk:kk + 1])

            def ln_apply_group(c, mun, std, g_t, b_t, outw=None):
                rstd, nmu = ln_finish(mun, std)
                for kk in range(4):
                    nt = 4 * c + kk
                    dst = (outw[:, kk * D:(kk + 1) * D]
                           if outw is not None else H[nt][:])
                    if kk % 2 == 0 and outw is None:
                        nc.scalar.activation(
                            dst, H[nt][:], AF.Identity,
                            scale=rstd[:, kk:kk + 1],
                            bias=nmu[:, kk:kk + 1])
                    else:
                        nc.vector.tensor_scalar(
                            out=dst, in0=H[nt][:],
                            scalar1=rstd[:, kk:kk + 1],
                            scalar2=nmu[:, kk:kk + 1],
                            op0=OP.mult, op1=OP.add)
                    if g_t is not None:
                        nc.vector.tensor_tensor(out=dst, in0=dst,
                                                in1=g_t[:], op=OP.mult)
                        nc.vector.tensor_tensor(out=dst, in0=dst,
                                                in1=b_t[:], op=OP.add)

            def av_chunk(l, c):
                """Attention update + LN1 for the 4 row tiles of chunk c."""
                sum_g = smallp.tile([P, 4], f32, tag="sumg", name="sumg")
                ssq_g = smallp.tile([P, 4], f32, tag="ssqg", name="ssqg")
                for ib_l in range(4):
                    ib = 4 * c + ib_l
                    av = pavp.tile([P, D], f32, tag="pav", name="pav")
                    for jt in range(ib + 1):
                        nc.tensor.matmul(
                            av[:],
                            ST[c, jt][:, ib_l * P:(ib_l + 1) * P],
                            Vsl(jt),
                            start=(jt == 0), stop=(jt == ib),
                        )
                    resid_stats(ib, av, sum_g, ssq_g, ib_l,
                                scale=invpos[:, ib:ib + 1])
                mun, std = ln_group_stats(sum_g, ssq_g, 4)
                # deferred: caller emits the applies after the next chunk's
                # residuals so they don't head-of-line block the DVE queue
                return lambda: ln_apply_group(
                    c, mun, std,
                    ln1g_t[l] if use_ln1_gb else None,
                    ln1b_t[l] if use_ln1_gb else None)

            def mlp1_chunk(l, c):
                for et in range(DT):
                    ps = pqp.tile([P, CW], f32, tag="pq", name="ps_h1")
                    for dt_ in range(DT):
                        nc.tensor.matmul(
                            ps[:],
                            W["w1", l, dt_][:, et * P:(et + 1) * P],
                            HTm[dt_][:, c * CW:(c + 1) * CW],
                            start=(dt_ == 0), stop=(dt_ == DT - 1),
                            skip_group_check=True,
                        )
                    if use_b1:
                        nc.scalar.activation(
                            h1T[et][:, c * CW:(c + 1) * CW], ps[:],
                            AF.Relu, bias=b1_t[l][:, et:et + 1])
                    else:
                        nc.scalar.activation(
                            h1T[et][:, c * CW:(c + 1) * CW], ps[:], AF.Relu)

            def mlp2_chunk(l, c, final=False):
                sum_g = smallp.tile([P, 4], f32, tag="sumg", name="sumg2")
                ssq_g = smallp.tile([P, 4], f32, tag="ssqg", name="ssqg2")
                for kk in range(4):
                    nt = 4 * c + kk
                    ps = pavp.tile([P, D], f32, tag="pav", name="ps_m")
                    for et in range(DT):
                        nc.tensor.matmul(
                            ps[:],
                            h1T[et][:, nt * P:(nt + 1) * P],
                            W["w2", l, et][:],
                            start=(et == 0), stop=(et == DT - 1),
                        )
                    if use_b2:
                        nc.vector.scalar_tensor_tensor(
                            out=H[nt][:], in0=ps[:], scalar=1.0,
                            in1=H[nt][:], op0=OP.mult, op1=OP.add)
                        nc.vector.tensor_tensor(out=H[nt][:], in0=H[nt][:],
                                                in1=b2_t[l][:], op=OP.add)
                        nc.vector.tensor_reduce(
                            out=sum_g[:, kk:kk + 1], in_=H[nt][:],
                            axis=mybir.AxisListType.X, op=OP.add)
                        sq = sqp.tile([P, D], bf16, tag="sq", name="sq2")
                        nc.gpsimd.scalar_tensor_tensor(
                            out=sq[:], in0=H[nt][:], scalar=1.0,
                            in1=H[nt][:], op0=OP.mult, op1=OP.mult,
                            accum_out=ssq_g[:, kk:kk + 1])
                    else:
                        resid_stats(nt, ps, sum_g, ssq_g, kk)
                mun, std = ln_group_stats(sum_g, ssq_g, 4)

                def apply_thunk():
                    g_t = ln2g_t[l] if use_ln2_gb else None
                    b_t = ln2b_t[l] if use_ln2_gb else None
                    if final:
                        # final LN writes a contiguous staging tile (all on
                        # DVE) so the output leaves as one 4-tile DMA
                        outw = workp.tile([P, 4 * D], bf16, tag=f"outw_{c}",
                                          name=f"outw_{c}")
                        ln_apply_group(c, mun, std, g_t, b_t, outw=outw)
                        nc.sync.dma_start(
                            out_d[4 * c * P:(4 * c + 4) * P, :].rearrange(
                                "(nt p) d -> p nt d", p=P),
                            outw.rearrange("p (nt d) -> p nt d", nt=4))
                    else:
                        ln_apply_group(c, mun, std, g_t, b_t)
                return apply_thunk

            def phaseA_steps(l):
                """Next-layer attn prep as a list of emission thunks."""
                return [
                    lambda: transpose_chunk(HTa, 0),
                    lambda: qproj_chunk(l, 0),
                    lambda: transpose_chunk(HTa, 1),
                    lambda: vproj_chunk(l, 0),
                    lambda: qproj_chunk(l, 1),
                    lambda: transpose_chunk(HTa, 2),
                    lambda: vproj_chunk(l, 1),
                    lambda: qproj_chunk(l, 2),
                    lambda: transpose_chunk(HTa, 3),
                    lambda: vproj_chunk(l, 2),
                    lambda: qproj_chunk(l, 3),
                    lambda: vproj_chunk(l, 3),
                ]

            # ---------------- layer loop ----------------
            # layer-0 phase A prologue: HTa comes from the transposed h0 DMA,
            # so only q~ and V projections are needed.
            for c in range(CH):
                qproj_chunk(0, c)
                vproj_chunk(0, c)

            NL = L * REPEAT
            for li in range(NL):
                l = li % L
                ln = (li + 1) % L  # next layer's weights
                fin = li == NL - 1
                # phase B: scores one chunk ahead of AV; LN applies deferred
                # one chunk (DVE head-of-line); MLP transposes (HTm) slot in
                # behind their LN1.
                scores_chunk(0)
                scores_chunk(1)
                apA0 = av_chunk(l, 0)
                scores_chunk(2)
                apA1 = av_chunk(l, 1)
                apA0()
                transpose_chunk(HTm, 0)
                scores_chunk(3)
                apA2 = av_chunk(l, 2)
                apA1()
                transpose_chunk(HTm, 1)
                apA3 = av_chunk(l, 3)
                apA2()
                transpose_chunk(HTm, 2)
                # phase C: mlp1 ahead of mlp2; next layer's phase A slots in
                # behind LN2.
                mlp1_chunk(l, 0)
                apA3()
                mlp1_chunk(l, 1)
                transpose_chunk(HTm, 3)
                mlp1_chunk(l, 2)
                apM0 = mlp2_chunk(l, 0, final=fin)
                mlp1_chunk(l, 3)
                apM1 = mlp2_chunk(l, 1, final=fin)
                apM0()
                apM2 = mlp2_chunk(l, 2, final=fin)
                apM1()
                apM3 = mlp2_chunk(l, 3, final=fin)
                apM2()
                if li < NL - 1:
                    (ta0, q0, ta1, v0, q1, ta2, v1, q2, ta3, v2, q3,
                     v3) = phaseA_steps(ln)
                    ta0()
                    q0()
                    apM3()
                    ta1()
                    v0()
                    q1()
                    ta2()
                    v1()
                    q2()
                    ta3()
                    v2()
                    q3()
                    v3()
                else:
                    apM3()

    nc.finalize()
    return nc


def kernel(**inputs):
    global LAST_EXEC_NS, LAST_RESULTS
    import ml_dtypes
    from concourse import bass_utils

    bf = ml_dtypes.bfloat16

    x = np.asarray(inputs["x"], dtype=np.float32)
    wpe = np.asarray(inputs["wpe"], dtype=np.float32)
    assert x.shape == (B, N, D), x.shape

    use_b1 = bool(np.any(np.asarray(inputs["mlp_b1"]) != 0))
    use_b2 = bool(np.any(np.asarray(inputs["mlp_b2"]) != 0))
    use_ln1 = not (np.all(np.asarray(inputs["ln1_g"]) == 1)
                   and np.all(np.asarray(inputs["ln1_b"]) == 0))
    use_ln2 = not (np.all(np.asarray(inputs["ln2_g"]) == 1)
                   and np.all(np.asarray(inputs["ln2_b"]) == 0))

    key = (use_b1, use_b2, use_ln1, use_ln2)
    if key not in _CACHE:
        _CACHE[key] = _build_program(*key)
    nc = _CACHE[key]

    h0 = x + wpe[None, :, :]  # positional embedding folded in on host

    Wq = np.asarray(inputs["Wq"], dtype=np.float32)
    Wk = np.asarray(inputs["Wk"], dtype=np.float32)
    M = np.einsum("lde,lfe->ldf", Wq, Wk)  # M[l] = Wq[l] @ Wk[l]^T

    ident = np.eye(P, dtype=np.float32)
    # trimask[k][jj, c] = 1 if c >= 128*k + jj else 0  (keep j <= i in-tile)
    jj = np.arange(P)[:, None]
    cc = np.arange(CW)[None, :]
    tri = np.stack([(cc >= P * k + jj) for k in range(4)]).astype(np.float32)
    pos = np.arange(N, dtype=np.float32).reshape(NT, P).T  # [P, NT]
    invpos = (1.0 / (pos + 1.0)).astype(np.float32)

    shared = {
        "ident": ident.astype(bf),
        "tri": tri.astype(bf),
        "invpos": invpos,
        "m": np.ascontiguousarray(M).astype(bf),
        "wv": np.ascontiguousarray(inputs["Wv"], dtype=np.float32).astype(bf),
        "w1": np.ascontiguousarray(inputs["mlp_W1"],
                                   dtype=np.float32).astype(bf),
        "w2": np.ascontiguousarray(inputs["mlp_W2"],
                                   dtype=np.float32).astype(bf),
    }
    if use_b1:
        shared["b1"] = np.asarray(inputs["mlp_b1"], dtype=np.float32)
    if use_b2:
        shared["b2"] = np.asarray(inputs["mlp_b2"],
                                  dtype=np.float32).astype(bf)
    if use_ln1:
        shared["ln1g"] = np.asarray(inputs["ln1_g"],
                                    dtype=np.float32).astype(bf)
        shared["ln1b"] = np.asarray(inputs["ln1_b"],
                                    dtype=np.float32).astype(bf)
    if use_ln2:
        shared["ln2g"] = np.asarray(inputs["ln2_g"],
                                    dtype=np.float32).astype(bf)
        shared["ln2b"] = np.asarray(inputs["ln2_b"],
                                    dtype=np.float32).astype(bf)

    in_maps = [dict(shared,
                    h0=np.ascontiguousarray(h0[c]).astype(bf),
                    h0t=np.ascontiguousarray(h0[c].T).astype(bf))
               for c in range(B)]
    global _last_in_maps
    _last_in_maps = in_maps

    res = bass_utils.run_bass_kernel_spmd(
        nc, in_maps, core_ids=list(range(B)), trace=PROFILE)
    LAST_EXEC_NS = res.exec_time_ns
    LAST_RESULTS = res
    return np.stack([np.asarray(res.results[c]["out"]).astype(np.float32)
                     for c in range(B)], axis=0)


# revision 52
# speedup vs baseline: 1.4263x; 1.0488x over previous
"""Trainium2 Bass kernel for nn_DecoderTransformerBackbone_1589137900084.

Decoder transformer backbone: B=8, N=2048, D=256, L=4 layers of
relu-attention with a causal averaging mask + MLP, layernorms after each
residual. Data-parallel over batch: one batch element per NeuronCore (8 cores).

Design (v3):
  - All matmul operands are bf16 (1 cyc/row on PE incl. transposes); the
    residual stream H is bf16; PSUM accumulation is fp32. Numpy emulation of
    this quantization gives rel_err ~1.1e-2 < 2e-2.
  - Wk is folded into Wq on the host: M = Wq @ Wk^T, so scores are
    S = (H M) H^T and the k projection disappears.
  - The causal averaging scale 1/(i+1) is folded into the q~T PSUM->SBUF
    copy (elementwise multiply with a broadcast inverse-position row), so
    the attention residual update is a plain fused add.
  - Elementwise split: Act takes wide PSUM relu/copies (V, full score tiles,
    h1), DVE takes HT copies, q~ scale, diagonal score tiles, residual adds
    (fused row-sum) and sum-of-squares (4x bf16 mode), Pool (gpsimd) takes
    the SBUF-only LN applies (except tail chunks, which go to DVE to
    shorten the critical chain).
  - Full software pipelining: HT is double-buffered (attn vs MLP), MLP
    transposes interleave into the attention phase, and the next layer's
    phase A (transpose + q~ + V) interleaves into this layer's MLP phase,
    so PE never waits for an LN tail.
"""
import sys

sys.path.insert(0, "/opt/trn_rl_repo")

import numpy as np

B, N, D, L = 8, 2048, 256, 4
LN_EPS = 1e-5
P = 128
NT = N // P            # 16 token tiles
DT = D // P            # 2 dim tiles
CH = 4                 # chunks of 512 tokens
CW = N // CH           # 512

_CACHE = {}
_last_in_maps = None
REPEAT = 1
PROFILE = False
LAST_EXEC_NS = None
LAST_RESULTS = None


def _build_program(use_b1, use_b2, use_ln1_gb, use_ln2_gb):
    import concourse.bass as bass  # noqa: F401
    from concourse import bacc
    import concourse.mybir as mybir
    import concourse.tile as tile

    f32 = mybir.dt.float32
    bf16 = mybir.dt.bfloat16
    AF = mybir.ActivationFunctionType
    OP = mybir.AluOpType

    nc = bacc.Bacc("TRN2", target_bir_lowering=False)

    ident_d = nc.declare_dram_parameter("ident", [P, P], bf16, isOutput=False)
    tri_d = nc.declare_dram_parameter("tri", [4, P, CW], bf16, isOutput=False)
    invpos_d = nc.declare_dram_parameter("invpos", [P, NT], f32,
                                         isOutput=False)
    h0_d = nc.declare_dram_parameter("h0", [N, D], bf16, isOutput=False)
    h0t_d = nc.declare_dram_parameter("h0t", [D, N], bf16, isOutput=False)
    m_d = nc.declare_dram_parameter("m", [L, D, D], bf16, isOutput=False)
    wv_d = nc.declare_dram_parameter("wv", [L, D, D], bf16, isOutput=False)
    w1_d = nc.declare_dram_parameter("w1", [L, D, D], bf16, isOutput=False)
    w2_d = nc.declare_dram_parameter("w2", [L, D, D], bf16, isOutput=False)
    if use_b1:
        b1_d = nc.declare_dram_parameter("b1", [L, D], f32, isOutput=False)
    if use_b2:
        b2_d = nc.declare_dram_parameter("b2", [L, D], bf16, isOutput=False)
    if use_ln1_gb:
        ln1g_d = nc.declare_dram_parameter("ln1g", [L, D], bf16, isOutput=False)
        ln1b_d = nc.declare_dram_parameter("ln1b", [L, D], bf16, isOutput=False)
    if use_ln2_gb:
        ln2g_d = nc.declare_dram_parameter("ln2g", [L, D], bf16, isOutput=False)
        ln2b_d = nc.declare_dram_parameter("ln2b", [L, D], bf16, isOutput=False)
    out_d = nc.declare_dram_parameter("out", [N, D], bf16, isOutput=True)

    with tile.TileContext(nc) as tc:
        with (
            tc.tile_pool(name="const", bufs=1) as constp,
            tc.tile_pool(name="work", bufs=1) as workp,
            tc.tile_pool(name="stp", bufs=40) as stp,
            tc.tile_pool(name="sqp", bufs=3) as sqp,
            tc.tile_pool(name="small", bufs=24) as smallp,
            tc.tile_pool(name="pq", bufs=2, space="PSUM") as pqp,
            tc.tile_pool(name="psc", bufs=2, space="PSUM") as pscore,
            tc.tile_pool(name="pav", bufs=4, space="PSUM") as pavp,
        ):
            # ---------------- constants, inputs, weights ----------------
            # DMA order matters for startup: SP queue carries the layer-0
            # critical path (transposed h0, invpos, first H tiles, masks);
            # the Act queue carries weights in layer order.
            HTa = [workp.tile([P, N], bf16, tag=f"hta_{d}", name=f"hta_{d}")
                   for d in range(DT)]
            invpos = constp.tile([P, NT], f32, tag="invpos")
            tri = constp.tile([P, 4 * CW], bf16, tag="tri")
            HN = N // 2
            # first halves of h0^T + invpos + masks: layer-0 critical path
            for dt_ in range(DT):
                nc.sync.dma_start(HTa[dt_][:, :HN],
                                  h0t_d[dt_ * P:(dt_ + 1) * P, :HN])
            nc.sync.dma_start(invpos[:], invpos_d[:])
            for k in range(4):
                nc.sync.dma_start(tri[:, k * CW:(k + 1) * CW], tri_d[k])
            for dt_ in range(DT):
                nc.sync.dma_start(HTa[dt_][:, HN:],
                                  h0t_d[dt_ * P:(dt_ + 1) * P, HN:])
            H = []
            for nt in range(NT):
                t = workp.tile([P, D], bf16, tag=f"h_{nt}")
                nc.sync.dma_start(t[:], h0_d[nt * P:(nt + 1) * P, :])
                H.append(t)

            W = {}

            def load_weights(l, eng):
                for wname, wd in (("m", m_d), ("wv", wv_d), ("w1", w1_d),
                                  ("w2", w2_d)):
                    for dt_ in range(DT):
                        t = constp.tile([P, D], bf16, tag=f"{wname}_{l}_{dt_}")
                        eng.dma_start(
                            t[:], wd[l, dt_ * P:(dt_ + 1) * P, :])
                        W[wname, l, dt_] = t

            ident = constp.tile([P, P], bf16, tag="ident")
            eps_t = constp.tile([P, 1], f32, tag="eps")
            # m/wv for layer 0 race the prologue on the otherwise-idle Act
            # queue; everything else goes behind the h0 loads on SP.
            for wname, wd in (("m", m_d), ("wv", wv_d)):
                for dt_ in range(DT):
                    t = constp.tile([P, D], bf16, tag=f"{wname}_0_{dt_}")
                    nc.scalar.dma_start(t[:], wd[0, dt_ * P:(dt_ + 1) * P, :])
                    W[wname, 0, dt_] = t
            for wname, wd in (("w1", w1_d), ("w2", w2_d)):
                for dt_ in range(DT):
                    t = constp.tile([P, D], bf16, tag=f"{wname}_0_{dt_}")
                    nc.sync.dma_start(t[:], wd[0, dt_ * P:(dt_ + 1) * P, :])
                    W[wname, 0, dt_] = t
            nc.sync.dma_start(ident[:], ident_d[:])
            for l in range(1, L):
                load_weights(l, nc.sync)
            nc.vector.memset(eps_t[:], LN_EPS)

            def trimask(k):
                return tri[:, k * CW:(k + 1) * CW]

            def load_vec_f32(dram, tag):
                out = []
                for l in range(L):
                    t = constp.tile([P, DT], f32, tag=f"{tag}_{l}")
                    nc.sync.dma_start(
                        t[:], dram[l].rearrange("(dt p) -> p dt", p=P))
                    out.append(t)
                return out

            def load_bcast(dram, tag):
                out = []
                for l in range(L):
                    t = constp.tile([P, D], bf16, tag=f"{tag}b_{l}")
                    nc.sync.dma_start(
                        t[:], dram[l].unsqueeze(0).to_broadcast([P, D]))
                    out.append(t)
                return out

            b1_t = load_vec_f32(b1_d, "b1") if use_b1 else None
            b2_t = load_bcast(b2_d, "b2") if use_b2 else None
            ln1g_t = load_bcast(ln1g_d, "ln1g") if use_ln1_gb else None
            ln1b_t = load_bcast(ln1b_d, "ln1b") if use_ln1_gb else None
            ln2g_t = load_bcast(ln2g_d, "ln2g") if use_ln2_gb else None
            ln2b_t = load_bcast(ln2b_d, "ln2b") if use_ln2_gb else None

            # HTm: MLP-phase transpose of H (HTa declared above, DMA-seeded)
            HTm = [workp.tile([P, N], bf16, tag=f"htm_{d}", name=f"htm_{d}")
                   for d in range(DT)]
            qT = [workp.tile([P, N], bf16, tag=f"qt_{d}", name=f"qt_{d}")
                  for d in range(DT)]
            h1T = [workp.tile([P, N], bf16, tag=f"h1t_{d}", name=f"h1t_{d}")
                   for d in range(DT)]
            Vp = [workp.tile([P, 2 * D], bf16, tag=f"vp_{i}", name=f"vp_{i}")
                  for i in range(NT // 2)]

            def Vsl(nt):
                return Vp[nt // 2][:, (nt % 2) * D:(nt % 2 + 1) * D]

            # ---------------- helpers ----------------
            def transpose_chunk(HT, c):
                """PE-transpose H tiles of chunk c into HT[dt][:, c*512:...]."""
                ps = pscore.tile([P, 2 * CW], bf16, tag="psc", name="ps_tr")
                for dt_ in range(DT):
                    for k in range(4):
                        nc.tensor.transpose(
                            ps[:, dt_ * CW + k * P: dt_ * CW + (k + 1) * P],
                            H[4 * c + k][:, dt_ * P:(dt_ + 1) * P],
                            ident[:],
                        )
                for dt_ in range(DT):
                    nc.vector.tensor_copy(
                        HT[dt_][:, c * CW:(c + 1) * CW],
                        ps[:, dt_ * CW:(dt_ + 1) * CW])

            def qproj_chunk(l, c):
                """q~T chunk: qT[et][:, cW:] = M^T H^T (plain Act copy)."""
                for et in range(DT):
                    ps = pqp.tile([P, CW], f32, tag="pq", name="ps_q")
                    for dt_ in range(DT):
                        nc.tensor.matmul(
                            ps[:],
                            W["m", l, dt_][:, et * P:(et + 1) * P],
                            HTa[dt_][:, c * CW:(c + 1) * CW],
                            start=(dt_ == 0), stop=(dt_ == DT - 1),
                            skip_group_check=True,
                        )
                    nc.scalar.activation(
                        qT[et][:, c * CW:(c + 1) * CW], ps[:], AF.Copy)

            def vproj_chunk(l, c):
                """V for the two tile-pairs of chunk c (normal layout)."""
                for pr in range(2):
                    pair = 2 * c + pr
                    ps = pqp.tile([P, CW], f32, tag="pq", name="ps_v")
                    for k in range(2):
                        nt = 2 * pair + k
                        for dt_ in range(DT):
                            nc.tensor.matmul(
                                ps[:, k * D:(k + 1) * D],
                                HTa[dt_][:, nt * P:(nt + 1) * P],
                                W["wv", l, dt_][:],
                                start=(dt_ == 0), stop=(dt_ == DT - 1),
                                skip_group_check=True,
                            )
                    nc.scalar.activation(Vp[pair][:], ps[:], AF.Copy)

            ST = {}

            relu_rr = [0]

            def scores_chunk(c):
                """S~^T tiles for chunk c: jt in [0, 4c+3]. Score PSUM
                alternates between psc and the phase-B-idle pq pool (4
                effective bufs); full-tile relus rotate Act/Act/DVE."""
                for jt in range(4 * c + 4):
                    k = jt - 4 * c
                    c0 = P * max(0, k)
                    pool, ptag = ((pscore, "psc") if jt % 2 == 0
                                  else (pqp, "pq"))
                    ps = pool.tile([P, CW], f32, tag=ptag, name="ps_s")
                    for et in range(DT):
                        nc.tensor.matmul(
                            ps[:, c0:],
                            HTa[et][:, jt * P:(jt + 1) * P],
                            qT[et][:, c * CW + c0:(c + 1) * CW],
                            start=(et == 0), stop=(et == DT - 1),
                        )
                    st = stp.tile([P, CW], bf16, tag="st", name="st")
                    if k >= 0:
                        # diagonal tile: relu + in-tile triangle mask in one
                        # DVE op (stt max,mult -- HW-validated)
                        nc.vector.scalar_tensor_tensor(
                            out=st[:, c0:], in0=ps[:, c0:], scalar=0.0,
                            in1=trimask(k)[:, c0:], op0=OP.max, op1=OP.mult)
                    elif relu_rr[0] % 5 == 4:
                        nc.vector.tensor_scalar_max(st[:], ps[:], 0.0)
                        relu_rr[0] += 1
                    else:
                        nc.scalar.activation(st[:], ps[:], AF.Relu)
                        relu_rr[0] += 1
                    ST[c, jt] = st

            def ln_group_stats(sum_g, ssq_g, n):
                # std = sqrt((ssq - D*mu^2)/D + eps); rstd/nmu are computed
                # lazily in the deferred apply (after the Act sqrt) so no
                # DVE-queue op ever waits cross-engine.
                mun = smallp.tile([P, 4], f32, tag="lnmu", name="lnmu")
                t1 = smallp.tile([P, 4], f32, tag="lnt1", name="lnt1")
                std = smallp.tile([P, 4], f32, tag="lnstd", name="lnstd")
                nc.vector.tensor_scalar(
                    out=mun[:, :n], in0=sum_g[:, :n], scalar1=-1.0 / D,
                    scalar2=0.0, op0=OP.mult, op1=OP.add)
                nc.vector.tensor_tensor(out=t1[:, :n], in0=mun[:, :n],
                                        in1=mun[:, :n], op=OP.mult)
                nc.vector.scalar_tensor_tensor(
                    out=t1[:, :n], in0=t1[:, :n], scalar=-float(D),
                    in1=ssq_g[:, :n], op0=OP.mult, op1=OP.add)
                nc.scalar.activation(std[:, :n], t1[:, :n], AF.Sqrt,
                                     bias=eps_t[:], scale=1.0 / D)
                return mun, std

            def ln_finish(mun, std, n=4):
                # rstd = 1/std (DVE); nmu = mun * rstd (DVE, in-order)
                rstd = smallp.tile([P, 4], f32, tag="lnrstd", name="lnrstd")
                nmu = smallp.tile([P, 4], f32, tag="lnnmu", name="lnnmu")
                nc.vector.reciprocal(rstd[:, :n], std[:, :n])
                nc.vector.scalar_tensor_tensor(
                    out=nmu[:, :n], in0=mun[:, :n], scalar=1.0,
                    in1=rstd[:, :n], op0=OP.mult, op1=OP.mult)
                return rstd, nmu

            def resid_stats(nt, ps, sum_g, ssq_g, kk, scale=None,
                            act_sq=False):
                """H[nt] += scale*ps (PSUM); fused row-sum + sumsq."""
                nc.vector.scalar_tensor_tensor(
                    out=H[nt][:], in0=ps[:],
                    scalar=(1.0 if scale is None else scale), in1=H[nt][:],
                    op0=OP.mult, op1=OP.add,
                    accum_out=sum_g[:, kk:kk + 1])
                sq = sqp.tile([P, D], bf16, tag="sq", name="sq")
                if act_sq:
                    nc.scalar.activation(sq[:], H[nt][:], AF.Square,
                                         accum_out=ssq_g[:, kk:kk + 1])
                else:
                    nc.vector.scalar_tensor_tensor(
                        out=sq[:], in0=H[nt][:], scalar=1.0, in1=H[nt][:],
                        op0=OP.mult, op1=OP.mult,
                        accum_out=ssq_g[:, kk:kk + 1])

            def ln_apply_group(c, mun, std, g_t, b_t, outw=None):
                rstd, nmu = ln_finish(mun, std)
                for kk in range(4):
                    nt = 4 * c + kk
                    dst = (outw[:, kk * D:(kk + 1) * D]
                           if outw is not None else H[nt][:])
                    if kk % 2 == 0:
                        nc.scalar.activation(
                            dst, H[nt][:], AF.Identity,
                            scale=rstd[:, kk:kk + 1],
                            bias=nmu[:, kk:kk + 1])
                    else:
                        nc.vector.tensor_scalar(
                            out=dst, in0=H[nt][:],
                            scalar1=rstd[:, kk:kk + 1],
                            scalar2=nmu[:, kk:kk + 1],
                            op0=OP.mult, op1=OP.add)
                    if g_t is not None:
                        nc.vector.tensor_tensor(out=dst, in0=dst,
                                                in1=g_t[:], op=OP.mult)
                        nc.vector.tensor_tensor(out=dst, in0=dst,
                                                in1=b_t[:], op=OP.add)

            def av_chunk(l, c):
                """Attention update + LN1 for the 4 row tiles of chunk c."""
                sum_g = smallp.tile([P, 4], f32, tag="sumg", name="sumg")
                ssq_g = smallp.tile([P, 4], f32, tag="ssqg", name="ssqg")
                for ib_l in range(4):
                    ib = 4 * c + ib_l
                    av = pavp.tile([P, D], f32, tag="pav", name="pav")
                    for jt in range(ib + 1):
                        nc.tensor.matmul(
                            av[:],
                            ST[c, jt][:, ib_l * P:(ib_l + 1) * P],
                            Vsl(jt),
                            start=(jt == 0), stop=(jt == ib),
                        )
                    resid_stats(ib, av, sum_g, ssq_g, ib_l,
                                scale=invpos[:, ib:ib + 1])
                mun, std = ln_group_stats(sum_g, ssq_g, 4)
                # deferred: caller emits the applies after the next chunk's
                # residuals so they don't head-of-line block the DVE queue
                return lambda: ln_apply_group(
                    c, mun, std,
                    ln1g_t[l] if use_ln1_gb else None,
                    ln1b_t[l] if use_ln1_gb else None)

            def mlp1_chunk(l, c):
                for et in range(DT):
                    ps = pqp.tile([P, CW], f32, tag="pq", name="ps_h1")
                    for dt_ in range(DT):
                        nc.tensor.matmul(
                            ps[:],
                            W["w1", l, dt_][:, et * P:(et + 1) * P],
                            HTm[dt_][:, c * CW:(c + 1) * CW],
                            start=(dt_ == 0), stop=(dt_ == DT - 1),
                            skip_group_check=True,
                        )
                    if use_b1:
                        nc.scalar.activation(
                            h1T[et][:, c * CW:(c + 1) * CW], ps[:],
                            AF.Relu, bias=b1_t[l][:, et:et + 1])
                    else:
                        nc.scalar.activation(
                            h1T[et][:, c * CW:(c + 1) * CW], ps[:], AF.Relu)

            def mlp2_chunk(l, c, final=False):
                sum_g = smallp.tile([P, 4], f32, tag="sumg", name="sumg2")
                ssq_g = smallp.tile([P, 4], f32, tag="ssqg", name="ssqg2")
                for kk in range(4):
                    nt = 4 * c + kk
                    ps = pavp.tile([P, D], f32, tag="pav", name="ps_m")
                    for et in range(DT):
                        nc.tensor.matmul(
                            ps[:],
                            h1T[et][:, nt * P:(nt + 1) * P],
                            W["w2", l, et][:],
                            start=(et == 0), stop=(et == DT - 1),
                        )
                    if use_b2:
                        nc.vector.scalar_tensor_tensor(
                            out=H[nt][:], in0=ps[:], scalar=1.0,
                            in1=H[nt][:], op0=OP.mult, op1=OP.add)
                        nc.vector.tensor_tensor(out=H[nt][:], in0=H[nt][:],
                                                in1=b2_t[l][:], op=OP.add)
                        nc.vector.tensor_reduce(
                            out=sum_g[:, kk:kk + 1], in_=H[nt][:],
                            axis=mybir.AxisListType.X, op=OP.add)
                        sq = sqp.tile([P, D], bf16, tag="sq", name="sq2")
                        nc.gpsimd.scalar_tensor_tensor(
                            out=sq[:], in0=H[nt][:], scalar=1.0,
                            in1=H[nt][:], op0=OP.mult, op1=OP.mult,
                            accum_out=ssq_g[:, kk:kk + 1])
                    else:
                        resid_stats(nt, ps, sum_g, ssq_g, kk,
                                    act_sq=final and kk % 2 == 1)
                mun, std = ln_group_stats(sum_g, ssq_g, 4)

                def apply_thunk():
                    g_t = ln2g_t[l] if use_ln2_gb else None
                    b_t = ln2b_t[l] if use_ln2_gb else None
                    if final:
                        # final LN writes a contiguous staging tile (all on
                        # DVE) so the output leaves as one 4-tile DMA
                        outw = workp.tile([P, 4 * D], bf16, tag=f"outw_{c}",
                                          name=f"outw_{c}")
                        ln_apply_group(c, mun, std, g_t, b_t, outw=outw)
                        nc.sync.dma_start(
                            out_d[4 * c * P:(4 * c + 4) * P, :].rearrange(
                                "(nt p) d -> p nt d", p=P),
                            outw.rearrange("p (nt d) -> p nt d", nt=4))
                    else:
                        ln_apply_group(c, mun, std, g_t, b_t)
                return apply_thunk

            def phaseA_steps(l):
                """Next-layer attn prep as a list of emission thunks."""
                return [
                    lambda: transpose_chunk(HTa, 0),
                    lambda: qproj_chunk(l, 0),
                    lambda: transpose_chunk(HTa, 1),
                    lambda: vproj_chunk(l, 0),
                    lambda: qproj_chunk(l, 1),
                    lambda: transpose_chunk(HTa, 2),
                    lambda: vproj_chunk(l, 1),
                    lambda: qproj_chunk(l, 2),
                    lambda: transpose_chunk(HTa, 3),
                    lambda: vproj_chunk(l, 2),
                    lambda: qproj_chunk(l, 3),
                    lambda: vproj_chunk(l, 3),
                ]

            # ---------------- layer loop ----------------
            # Software-pipelined across layers: phase A of layer l+1 is
            # split -- chunks 0/1 (transpose+q~+V) emit at the tail of layer
            # l; chunks 2/3 weave into layer l+1's own phase B, so the DVE
            # residual/LN backlog from mlp2 drains under scores/AV cover.
            # layer-0 prologue: HTa is DMA-seeded from transposed h0.
            qproj_chunk(0, 0)
            vproj_chunk(0, 0)
            qproj_chunk(0, 1)
            vproj_chunk(0, 1)

            NL = L * REPEAT
            for li in range(NL):
                l = li % L
                ln = (li + 1) % L  # next layer's weights
                fin = li == NL - 1
                first = li == 0
                # fully software-pipelined body: MLP chunks start as soon as
                # their LN1 lands (under scores/AV cover); next layer's
                # phase-A chunks 0/1 fill the tail, chunks 2/3 weave into
                # the next body's own phase B.
                scores_chunk(0)
                if not first:
                    transpose_chunk(HTa, 2)
                scores_chunk(1)
                qproj_chunk(l, 2)
                if not first:
                    transpose_chunk(HTa, 3)
                apA0 = av_chunk(l, 0)
                qproj_chunk(l, 3)
                vproj_chunk(l, 2)
                scores_chunk(2)
                apA1 = av_chunk(l, 1)
                apA0()
                transpose_chunk(HTm, 0)
                vproj_chunk(l, 3)
                mlp1_chunk(l, 0)
                scores_chunk(3)
                apA2 = av_chunk(l, 2)
                apA1()
                transpose_chunk(HTm, 1)
                mlp1_chunk(l, 1)
                apM0 = mlp2_chunk(l, 0, final=fin)
                apA3 = av_chunk(l, 3)
                apA2()
                transpose_chunk(HTm, 2)
                mlp1_chunk(l, 2)
                apM1 = mlp2_chunk(l, 1, final=fin)
                apM0()
                apA3()
                transpose_chunk(HTm, 3)
                mlp1_chunk(l, 3)
                apM2 = mlp2_chunk(l, 2, final=fin)
                apM1()
                if li < NL - 1:
                    transpose_chunk(HTa, 0)
                    apM3 = mlp2_chunk(l, 3, final=fin)
                    apM2()
                    transpose_chunk(HTa, 1)
                    qproj_chunk(ln, 0)
                    apM3()
                    vproj_chunk(ln, 0)
                    qproj_chunk(ln, 1)
                    vproj_chunk(ln, 1)
                else:
                    apM3 = mlp2_chunk(l, 3, final=fin)
                    apM2()
                    apM3()

    nc.finalize()
    return nc


def kernel(**inputs):
    global LAST_EXEC_NS, LAST_RESULTS
    import ml_dtypes
    from concourse import bass_utils

    bf = ml_dtypes.bfloat16

    x = np.asarray(inputs["x"], dtype=np.float32)
    wpe = np.asarray(inputs["wpe"], dtype=np.float32)
    assert x.shape == (B, N, D), x.shape

    use_b1 = bool(np.any(np.asarray(inputs["mlp_b1"]) != 0))
    use_b2 = bool(np.any(np.asarray(inputs["mlp_b2"]) != 0))
    use_ln1 = not (np.all(np.asarray(inputs["ln1_g"]) == 1)
                   and np.all(np.asarray(inputs["ln1_b"]) == 0))
    use_ln2 = not (np.all(np.asarray(inputs["ln2_g"]) == 1)
                   and np.all(np.asarray(inputs["ln2_b"]) == 0))

    key = (use_b1, use_b2, use_ln1, use_ln2)
    if key not in _CACHE:
        _CACHE[key] = _build_program(*key)
    nc = _CACHE[key]

    h0 = x + wpe[None, :, :]  # positional embedding folded in on host

    Wq = np.asarray(inputs["Wq"], dtype=np.float32)
    Wk = np.asarray(inputs["Wk"], dtype=np.float32)
    M = np.einsum("lde,lfe->ldf", Wq, Wk)  # M[l] = Wq[l] @ Wk[l]^T

    ident = np.eye(P, dtype=np.float32)
    # trimask[k][jj, c] = 1 if c >= 128*k + jj else 0  (keep j <= i in-tile)
    jj = np.arange(P)[:, None]
    cc = np.arange(CW)[None, :]
    tri = np.stack([(cc >= P * k + jj) for k in range(4)]).astype(np.float32)
    pos = np.arange(N, dtype=np.float32).reshape(NT, P).T  # [P, NT]
    invpos = (1.0 / (pos + 1.0)).astype(np.float32)

    shared = {
        "ident": ident.astype(bf),
        "tri": tri.astype(bf),
        "invpos": invpos,
        "m": np.ascontiguousarray(M).astype(bf),
        "wv": np.ascontiguousarray(inputs["Wv"], dtype=np.float32).astype(bf),
        "w1": np.ascontiguousarray(inputs["mlp_W1"],
                                   dtype=np.float32).astype(bf),
        "w2": np.ascontiguousarray(inputs["mlp_W2"],
                                   dtype=np.float32).astype(bf),
    }
    if use_b1:
        shared["b1"] = np.asarray(inputs["mlp_b1"], dtype=np.float32)
    if use_b2:
        shared["b2"] = np.asarray(inputs["mlp_b2"],
                                  dtype=np.float32).astype(bf)
    if use_ln1:
        shared["ln1g"] = np.asarray(inputs["ln1_g"],
                                    dtype=np.float32).astype(bf)
        shared["ln1b"] = np.asarray(inputs["ln1_b"],
                                    dtype=np.float32).astype(bf)
    if use_ln2:
        shared["ln2g"] = np.asarray(inputs["ln2_g"],
                                    dtype=np.float32).astype(bf)
        shared["ln2b"] = np.asarray(inputs["ln2_b"],
                                    dtype=np.float32).astype(bf)

    in_maps = [dict(shared,
                    h0=np.ascontiguousarray(h0[c]).astype(bf),
                    h0t=np.ascontiguousarray(h0[c].T).astype(bf))
               for c in range(B)]
    global _last_in_maps
    _last_in_maps = in_maps

    res = bass_utils.run_bass_kernel_spmd(
        nc, in_maps, core_ids=list(range(B)), trace=PROFILE)
    LAST_EXEC_NS = res.exec_time_ns
    LAST_RESULTS = res
    return np.stack([np.asarray(res.results[c]["out"]).astype(np.float32)
                     for c in range(B)], axis=0)


# revision 67
# speedup vs baseline: 1.5150x; 1.0622x over previous
"""Trainium2 Bass kernel for nn_DecoderTransformerBackbone_1589137900084.

Decoder transformer backbone: B=8, N=2048, D=256, L=4 layers of
relu-attention with a causal averaging mask + MLP, layernorms after each
residual. Data-parallel over batch: one batch element per NeuronCore (8
cores). Measured on HW: rel_err 1.218e-2 (< 2e-2), 221.4 us vs 335.2 us
for the fp32r baseline (1.51x).

Design:
  - All matmul operands are bf16 (1 cyc/row on PE, incl. transposes at half
    the fp32 cost); the residual stream H is bf16; PSUM accumulates fp32.
  - Wk is folded into Wq on the host: M = Wq @ Wk^T, so scores are
    S = (H M) H^T and the k projection disappears (saves 8k PE cyc/layer
    plus its PSUM->SBUF copies). h0 = x + wpe and its transpose h0t are
    host-computed, so layer 0 needs no PE transposes.
  - The 1/(i+1) causal-average scale rides the [P,1] scalar slot of the
    fused residual-add (scalar_tensor_tensor), which also row-sum
    accumulates for the LN mean. A second DVE op accumulates sum-of-
    squares; LN std is sqrt on Act; 1/std + applies are deferred one chunk
    so nothing in the DVE queue ever waits cross-engine (head-of-line).
  - Engine split (all combos HW-validated; gpsimd cannot run
    TensorScalarPtr or touch PSUM): Act takes wide PSUM relu/copies
    (q~, half of V, h1, most score tiles) and half the LN applies; DVE
    takes HT copies, diagonal score tiles (relu+triangle mask in one stt
    max,mult), residual adds, sumsq, LN smalls, and the other applies.
  - PE-assisted residual for one mlp2 tile per chunk: W2 carries a 257th
    column of row-sums (psum col 256 = LN row-sum for free, valid because
    LN1 output sums to ~0) and H is identity-injected into the PSUM
    accumulation, so the writeback is a plain Act copy instead of a DVE
    tensor-tensor add.
  - Fully software-pipelined emission: MLP chunks start as soon as their
    LN1 lands (under scores/AV cover); the next layer's phase-A chunks 0/1
    fill the layer tail and chunks 2/3 weave into the next phase B. Score
    PSUM alternates between two pools (4 bufs effective); the final layer
    writes LN2 output into contiguous staging tiles so the result leaves
    as one 4-tile DMA per chunk.
"""
import sys

sys.path.insert(0, "/opt/trn_rl_repo")

import numpy as np

B, N, D, L = 8, 2048, 256, 4
LN_EPS = 1e-5
P = 128
NT = N // P            # 16 token tiles
DT = D // P            # 2 dim tiles
CH = 4                 # chunks of 512 tokens
CW = N // CH           # 512

_CACHE = {}
_last_in_maps = None
REPEAT = 1
PROFILE = False
LAST_EXEC_NS = None
LAST_RESULTS = None


def _build_program(use_b1, use_b2, use_ln1_gb, use_ln2_gb):
    import concourse.bass as bass  # noqa: F401
    from concourse import bacc
    import concourse.mybir as mybir
    import concourse.tile as tile

    f32 = mybir.dt.float32
    bf16 = mybir.dt.bfloat16
    AF = mybir.ActivationFunctionType
    OP = mybir.AluOpType

    nc = bacc.Bacc("TRN2", target_bir_lowering=False)

    ident_d = nc.declare_dram_parameter("ident", [P, P], bf16, isOutput=False)
    tri_d = nc.declare_dram_parameter("tri", [4, P, CW], bf16, isOutput=False)
    invpos_d = nc.declare_dram_parameter("invpos", [P, NT], f32,
                                         isOutput=False)
    h0_d = nc.declare_dram_parameter("h0", [N, D], bf16, isOutput=False)
    h0t_d = nc.declare_dram_parameter("h0t", [D, N], bf16, isOutput=False)
    m_d = nc.declare_dram_parameter("m", [L, D, D], bf16, isOutput=False)
    wv_d = nc.declare_dram_parameter("wv", [L, D, D], bf16, isOutput=False)
    w1_d = nc.declare_dram_parameter("w1", [L, D, D], bf16, isOutput=False)
    w2_d = nc.declare_dram_parameter("w2", [L, D, D + 1], bf16,
                                     isOutput=False)
    if use_b1:
        b1_d = nc.declare_dram_parameter("b1", [L, D], f32, isOutput=False)
    if use_b2:
        b2_d = nc.declare_dram_parameter("b2", [L, D], bf16, isOutput=False)
    if use_ln1_gb:
        ln1g_d = nc.declare_dram_parameter("ln1g", [L, D], bf16, isOutput=False)
        ln1b_d = nc.declare_dram_parameter("ln1b", [L, D], bf16, isOutput=False)
    if use_ln2_gb:
        ln2g_d = nc.declare_dram_parameter("ln2g", [L, D], bf16, isOutput=False)
        ln2b_d = nc.declare_dram_parameter("ln2b", [L, D], bf16, isOutput=False)
    out_d = nc.declare_dram_parameter("out", [N, D], bf16, isOutput=True)

    with tile.TileContext(nc) as tc:
        with (
            tc.tile_pool(name="const", bufs=1) as constp,
            tc.tile_pool(name="work", bufs=1) as workp,
            tc.tile_pool(name="stp", bufs=40) as stp,
            tc.tile_pool(name="sqp", bufs=3) as sqp,
            tc.tile_pool(name="small", bufs=24) as smallp,
            tc.tile_pool(name="pq", bufs=3, space="PSUM") as pqp,
            tc.tile_pool(name="psc", bufs=2, space="PSUM") as pscore,
            tc.tile_pool(name="pav", bufs=3, space="PSUM") as pavp,
        ):
            # ---------------- constants, inputs, weights ----------------
            # DMA order matters for startup: SP queue carries the layer-0
            # critical path (transposed h0, invpos, first H tiles, masks);
            # the Act queue carries weights in layer order.
            HTa = [workp.tile([P, N], bf16, tag=f"hta_{d}", name=f"hta_{d}")
                   for d in range(DT)]
            invpos = constp.tile([P, NT], f32, tag="invpos")
            tri = constp.tile([P, 4 * CW], bf16, tag="tri")
            HN = N // 2
            # first halves of h0^T + invpos + masks: layer-0 critical path
            for dt_ in range(DT):
                nc.sync.dma_start(HTa[dt_][:, :HN],
                                  h0t_d[dt_ * P:(dt_ + 1) * P, :HN])
            nc.sync.dma_start(invpos[:], invpos_d[:])
            for k in range(4):
                nc.sync.dma_start(tri[:, k * CW:(k + 1) * CW], tri_d[k])
            for dt_ in range(DT):
                nc.sync.dma_start(HTa[dt_][:, HN:],
                                  h0t_d[dt_ * P:(dt_ + 1) * P, HN:])
            H = []
            for nt in range(NT):
                t = workp.tile([P, D], bf16, tag=f"h_{nt}")
                nc.sync.dma_start(t[:], h0_d[nt * P:(nt + 1) * P, :])
                H.append(t)

            W = {}

            def load_weights(l, eng):
                for wname, wd in (("m", m_d), ("wv", wv_d), ("w1", w1_d),
                                  ("w2", w2_d)):
                    wdim = D + 1 if wname == "w2" else D
                    for dt_ in range(DT):
                        t = constp.tile([P, wdim], bf16,
                                        tag=f"{wname}_{l}_{dt_}")
                        eng.dma_start(
                            t[:], wd[l, dt_ * P:(dt_ + 1) * P, :])
                        W[wname, l, dt_] = t

            ident = constp.tile([P, P], bf16, tag="ident")
            eps_t = constp.tile([P, 1], f32, tag="eps")
            # m/wv for layer 0 race the prologue on the otherwise-idle Act
            # queue; everything else goes behind the h0 loads on SP.
            for wname, wd in (("m", m_d), ("wv", wv_d)):
                for dt_ in range(DT):
                    t = constp.tile([P, D], bf16, tag=f"{wname}_0_{dt_}")
                    nc.scalar.dma_start(t[:], wd[0, dt_ * P:(dt_ + 1) * P, :])
                    W[wname, 0, dt_] = t
            for wname, wd in (("w1", w1_d), ("w2", w2_d)):
                wdim = D + 1 if wname == "w2" else D
                for dt_ in range(DT):
                    t = constp.tile([P, wdim], bf16, tag=f"{wname}_0_{dt_}")
                    nc.sync.dma_start(t[:], wd[0, dt_ * P:(dt_ + 1) * P, :])
                    W[wname, 0, dt_] = t
            nc.sync.dma_start(ident[:], ident_d[:])
            for l in range(1, L):
                load_weights(l, nc.sync)
            nc.vector.memset(eps_t[:], LN_EPS)

            def trimask(k):
                return tri[:, k * CW:(k + 1) * CW]

            def load_vec_f32(dram, tag):
                out = []
                for l in range(L):
                    t = constp.tile([P, DT], f32, tag=f"{tag}_{l}")
                    nc.sync.dma_start(
                        t[:], dram[l].rearrange("(dt p) -> p dt", p=P))
                    out.append(t)
                return out

            def load_bcast(dram, tag):
                out = []
                for l in range(L):
                    t = constp.tile([P, D], bf16, tag=f"{tag}b_{l}")
                    nc.sync.dma_start(
                        t[:], dram[l].unsqueeze(0).to_broadcast([P, D]))
                    out.append(t)
                return out

            b1_t = load_vec_f32(b1_d, "b1") if use_b1 else None
            b2_t = load_bcast(b2_d, "b2") if use_b2 else None
            ln1g_t = load_bcast(ln1g_d, "ln1g") if use_ln1_gb else None
            ln1b_t = load_bcast(ln1b_d, "ln1b") if use_ln1_gb else None
            ln2g_t = load_bcast(ln2g_d, "ln2g") if use_ln2_gb else None
            ln2b_t = load_bcast(ln2b_d, "ln2b") if use_ln2_gb else None

            # HTm: MLP-phase transpose of H (HTa declared above, DMA-seeded)
            HTm = [workp.tile([P, N], bf16, tag=f"htm_{d}", name=f"htm_{d}")
                   for d in range(DT)]
            qT = [workp.tile([P, N], bf16, tag=f"qt_{d}", name=f"qt_{d}")
                  for d in range(DT)]
            h1T = [workp.tile([P, N], bf16, tag=f"h1t_{d}", name=f"h1t_{d}")
                   for d in range(DT)]
            Vp = [workp.tile([P, 2 * D], bf16, tag=f"vp_{i}", name=f"vp_{i}")
                  for i in range(NT // 2)]

            def Vsl(nt):
                return Vp[nt // 2][:, (nt % 2) * D:(nt % 2 + 1) * D]

            # ---------------- helpers ----------------
            def transpose_chunk(HT, c):
                """PE-transpose H tiles of chunk c into HT[dt][:, c*512:...]."""
                ps = pscore.tile([P, 2 * CW], bf16, tag="psc", name="ps_tr")
                for dt_ in range(DT):
                    for k in range(4):
                        nc.tensor.transpose(
                            ps[:, dt_ * CW + k * P: dt_ * CW + (k + 1) * P],
                            H[4 * c + k][:, dt_ * P:(dt_ + 1) * P],
                            ident[:],
                        )
                for dt_ in range(DT):
                    nc.vector.tensor_copy(
                        HT[dt_][:, c * CW:(c + 1) * CW],
                        ps[:, dt_ * CW:(dt_ + 1) * CW])

            def qproj_chunk(l, c):
                """q~T chunk: qT[et][:, cW:] = M^T H^T (plain Act copy)."""
                for et in range(DT):
                    ps = pqp.tile([P, CW], f32, tag="pq", name="ps_q")
                    for dt_ in range(DT):
                        nc.tensor.matmul(
                            ps[:],
                            W["m", l, dt_][:, et * P:(et + 1) * P],
                            HTa[dt_][:, c * CW:(c + 1) * CW],
                            start=(dt_ == 0), stop=(dt_ == DT - 1),
                            skip_group_check=True,
                        )
                    nc.scalar.activation(
                        qT[et][:, c * CW:(c + 1) * CW], ps[:], AF.Copy)

            def vproj_chunk(l, c):
                """V for the two tile-pairs of chunk c (normal layout)."""
                for pr in range(2):
                    pair = 2 * c + pr
                    ps = pqp.tile([P, CW], f32, tag="pq", name="ps_v")
                    for k in range(2):
                        nt = 2 * pair + k
                        for dt_ in range(DT):
                            nc.tensor.matmul(
                                ps[:, k * D:(k + 1) * D],
                                HTa[dt_][:, nt * P:(nt + 1) * P],
                                W["wv", l, dt_][:],
                                start=(dt_ == 0), stop=(dt_ == DT - 1),
                                skip_group_check=True,
                            )
                    nc.scalar.activation(Vp[pair][:], ps[:], AF.Copy)

            ST = {}

            relu_rr = [0]

            def scores_chunk(c):
                """S~^T tiles for chunk c: jt in [0, 4c+3]. Score PSUM
                alternates between psc and the phase-B-idle pq pool (4
                effective bufs); full-tile relus rotate Act/Act/DVE."""
                for jt in range(4 * c + 4):
                    k = jt - 4 * c
                    c0 = P * max(0, k)
                    pool, ptag = ((pscore, "psc") if jt % 2 == 0
                                  else (pqp, "pq"))
                    ps = pool.tile([P, CW], f32, tag=ptag, name="ps_s")
                    for et in range(DT):
                        nc.tensor.matmul(
                            ps[:, c0:],
                            HTa[et][:, jt * P:(jt + 1) * P],
                            qT[et][:, c * CW + c0:(c + 1) * CW],
                            start=(et == 0), stop=(et == DT - 1),
                        )
                    st = stp.tile([P, CW], bf16, tag="st", name="st")
                    if k >= 0:
                        # diagonal tile: relu + in-tile triangle mask in one
                        # DVE op (stt max,mult -- HW-validated)
                        nc.vector.scalar_tensor_tensor(
                            out=st[:, c0:], in0=ps[:, c0:], scalar=0.0,
                            in1=trimask(k)[:, c0:], op0=OP.max, op1=OP.mult)
                    elif relu_rr[0] % 5 == 4:
                        nc.vector.tensor_scalar_max(st[:], ps[:], 0.0)
                        relu_rr[0] += 1
                    else:
                        nc.scalar.activation(st[:], ps[:], AF.Relu)
                        relu_rr[0] += 1
                    ST[c, jt] = st

            def ln_group_stats(sum_g, ssq_g, n):
                # std = sqrt((ssq - D*mu^2)/D + eps); rstd/nmu are computed
                # lazily in the deferred apply (after the Act sqrt) so no
                # DVE-queue op ever waits cross-engine.
                mun = smallp.tile([P, 4], f32, tag="lnmu", name="lnmu")
                t1 = smallp.tile([P, 4], f32, tag="lnt1", name="lnt1")
                std = smallp.tile([P, 4], f32, tag="lnstd", name="lnstd")
                nc.vector.tensor_scalar(
                    out=mun[:, :n], in0=sum_g[:, :n], scalar1=-1.0 / D,
                    scalar2=0.0, op0=OP.mult, op1=OP.add)
                nc.vector.tensor_tensor(out=t1[:, :n], in0=mun[:, :n],
                                        in1=mun[:, :n], op=OP.mult)
                nc.vector.scalar_tensor_tensor(
                    out=t1[:, :n], in0=t1[:, :n], scalar=-float(D),
                    in1=ssq_g[:, :n], op0=OP.mult, op1=OP.add)
                nc.scalar.activation(std[:, :n], t1[:, :n], AF.Sqrt,
                                     bias=eps_t[:], scale=1.0 / D)
                return mun, std

            def ln_finish(mun, std, n=4):
                # rstd = 1/std (DVE); nmu = mun * rstd (DVE, in-order)
                rstd = smallp.tile([P, 4], f32, tag="lnrstd", name="lnrstd")
                nmu = smallp.tile([P, 4], f32, tag="lnnmu", name="lnnmu")
                nc.vector.reciprocal(rstd[:, :n], std[:, :n])
                nc.vector.scalar_tensor_tensor(
                    out=nmu[:, :n], in0=mun[:, :n], scalar=1.0,
                    in1=rstd[:, :n], op0=OP.mult, op1=OP.mult)
                return rstd, nmu

            def resid_stats(nt, ps, sum_g, ssq_g, kk, scale=None,
                            act_sq=False):
                """H[nt] += scale*ps (PSUM); fused row-sum + sumsq."""
                nc.vector.scalar_tensor_tensor(
                    out=H[nt][:], in0=ps[:],
                    scalar=(1.0 if scale is None else scale), in1=H[nt][:],
                    op0=OP.mult, op1=OP.add,
                    accum_out=sum_g[:, kk:kk + 1])
                sq = sqp.tile([P, D], bf16, tag="sq", name="sq")
                if act_sq:
                    nc.scalar.activation(sq[:], H[nt][:], AF.Square,
                                         accum_out=ssq_g[:, kk:kk + 1])
                else:
                    nc.vector.scalar_tensor_tensor(
                        out=sq[:], in0=H[nt][:], scalar=1.0, in1=H[nt][:],
                        op0=OP.mult, op1=OP.mult,
                        accum_out=ssq_g[:, kk:kk + 1])

            def ln_apply_group(c, mun, std, g_t, b_t, outw=None):
                rstd, nmu = ln_finish(mun, std)
                for kk in range(4):
                    nt = 4 * c + kk
                    dst = (outw[:, kk * D:(kk + 1) * D]
                           if outw is not None else H[nt][:])
                    if kk % 2 == 0:
                        nc.scalar.activation(
                            dst, H[nt][:], AF.Identity,
                            scale=rstd[:, kk:kk + 1],
                            bias=nmu[:, kk:kk + 1])
                    else:
                        nc.vector.tensor_scalar(
                            out=dst, in0=H[nt][:],
                            scalar1=rstd[:, kk:kk + 1],
                            scalar2=nmu[:, kk:kk + 1],
                            op0=OP.mult, op1=OP.add)
                    if g_t is not None:
                        nc.vector.tensor_tensor(out=dst, in0=dst,
                                                in1=g_t[:], op=OP.mult)
                        nc.vector.tensor_tensor(out=dst, in0=dst,
                                                in1=b_t[:], op=OP.add)

            def av_chunk(l, c):
                """Attention update + LN1 for the 4 row tiles of chunk c."""
                sum_g = smallp.tile([P, 4], f32, tag="sumg", name="sumg")
                ssq_g = smallp.tile([P, 4], f32, tag="ssqg", name="ssqg")
                for ib_l in range(4):
                    ib = 4 * c + ib_l
                    av = pavp.tile([P, D], f32, tag="pav", name="pav")
                    for jt in range(ib + 1):
                        nc.tensor.matmul(
                            av[:],
                            ST[c, jt][:, ib_l * P:(ib_l + 1) * P],
                            Vsl(jt),
                            start=(jt == 0), stop=(jt == ib),
                        )
                    resid_stats(ib, av, sum_g, ssq_g, ib_l,
                                scale=invpos[:, ib:ib + 1])
                mun, std = ln_group_stats(sum_g, ssq_g, 4)
                # deferred: caller emits the applies after the next chunk's
                # residuals so they don't head-of-line block the DVE queue
                return lambda: ln_apply_group(
                    c, mun, std,
                    ln1g_t[l] if use_ln1_gb else None,
                    ln1b_t[l] if use_ln1_gb else None)

            def mlp1_chunk(l, c):
                for et in range(DT):
                    ps = pqp.tile([P, CW], f32, tag="pq", name="ps_h1")
                    for dt_ in range(DT):
                        nc.tensor.matmul(
                            ps[:],
                            W["w1", l, dt_][:, et * P:(et + 1) * P],
                            HTm[dt_][:, c * CW:(c + 1) * CW],
                            start=(dt_ == 0), stop=(dt_ == DT - 1),
                            skip_group_check=True,
                        )
                    if use_b1:
                        nc.scalar.activation(
                            h1T[et][:, c * CW:(c + 1) * CW], ps[:],
                            AF.Relu, bias=b1_t[l][:, et:et + 1])
                    else:
                        nc.scalar.activation(
                            h1T[et][:, c * CW:(c + 1) * CW], ps[:], AF.Relu)

            def mlp2_chunk(l, c, final=False):
                sum_g = smallp.tile([P, 4], f32, tag="sumg", name="sumg2")
                ssq_g = smallp.tile([P, 4], f32, tag="ssqg", name="ssqg2")
                sq_defer = []
                for kk in range(4):
                    nt = 4 * c + kk
                    # W2 carries a 257th column of row-sums, so psum col 256
                    # is the LN row-sum of the mlp2 output; for Act-writeback
                    # tiles, H is identity-injected into the accumulation on
                    # PE (LN1 output sums to ~0, so the column still gives
                    # the row-sum of H_new).
                    inject = (not use_b2) and kk % 2 == 0
                    ps = pavp.tile([P, D + 1], f32, tag="pav", name="ps_m")
                    nc.tensor.matmul(
                        ps[:], h1T[0][:, nt * P:(nt + 1) * P],
                        W["w2", l, 0][:], start=True, stop=False,
                        skip_group_check=True,
                    )
                    if inject:
                        nc.tensor.matmul(
                            ps[:, :D], ident[:], H[nt][:],
                            start=False, stop=False, skip_group_check=True,
                        )
                    nc.tensor.matmul(
                        ps[:], h1T[1][:, nt * P:(nt + 1) * P],
                        W["w2", l, 1][:], start=False, stop=True,
                        skip_group_check=True,
                    )
                    if use_b2:
                        nc.vector.scalar_tensor_tensor(
                            out=H[nt][:], in0=ps[:, :D], scalar=1.0,
                            in1=H[nt][:], op0=OP.mult, op1=OP.add)
                        nc.vector.tensor_tensor(out=H[nt][:], in0=H[nt][:],
                                                in1=b2_t[l][:], op=OP.add)
                        nc.vector.tensor_reduce(
                            out=sum_g[:, kk:kk + 1], in_=H[nt][:],
                            axis=mybir.AxisListType.X, op=OP.add)
                        sq = sqp.tile([P, D], bf16, tag="sq", name="sq2")
                        nc.gpsimd.scalar_tensor_tensor(
                            out=sq[:], in0=H[nt][:], scalar=1.0,
                            in1=H[nt][:], op0=OP.mult, op1=OP.mult,
                            accum_out=ssq_g[:, kk:kk + 1])
                    elif inject:
                        # writeback on Act; row-sum from the psum column
                        nc.scalar.activation(H[nt][:], ps[:, :D], AF.Copy)
                        nc.vector.tensor_copy(sum_g[:, kk:kk + 1],
                                              ps[:, D:D + 1])
                        sq_defer.append(nt)
                    else:
                        resid_stats(nt, ps[:, :D], sum_g, ssq_g, kk,
                                    act_sq=final and kk % 2 == 1)
                # deferred sumsq for Act-writeback tiles (emitted after all
                # residuals so the Act wait never blocks the DVE queue)
                for nt in sq_defer:
                    sq = sqp.tile([P, D], bf16, tag="sq", name="sq")
                    nc.vector.scalar_tensor_tensor(
                        out=sq[:], in0=H[nt][:], scalar=1.0, in1=H[nt][:],
                        op0=OP.mult, op1=OP.mult,
                        accum_out=ssq_g[:, nt % 4:nt % 4 + 1])
                mun, std = ln_group_stats(sum_g, ssq_g, 4)

                def apply_thunk():
                    g_t = ln2g_t[l] if use_ln2_gb else None
                    b_t = ln2b_t[l] if use_ln2_gb else None
                    if final:
                        # final LN writes a contiguous staging tile (all on
                        # DVE) so the output leaves as one 4-tile DMA
                        outw = workp.tile([P, 4 * D], bf16, tag=f"outw_{c}",
                                          name=f"outw_{c}")
                        ln_apply_group(c, mun, std, g_t, b_t, outw=outw)
                        nc.sync.dma_start(
                            out_d[4 * c * P:(4 * c + 4) * P, :].rearrange(
                                "(nt p) d -> p nt d", p=P),
                            outw.rearrange("p (nt d) -> p nt d", nt=4))
                    else:
                        ln_apply_group(c, mun, std, g_t, b_t)
                return apply_thunk

            def phaseA_steps(l):
                """Next-layer attn prep as a list of emission thunks."""
                return [
                    lambda: transpose_chunk(HTa, 0),
                    lambda: qproj_chunk(l, 0),
                    lambda: transpose_chunk(HTa, 1),
                    lambda: vproj_chunk(l, 0),
                    lambda: qproj_chunk(l, 1),
                    lambda: transpose_chunk(HTa, 2),
                    lambda: vproj_chunk(l, 1),
                    lambda: qproj_chunk(l, 2),
                    lambda: transpose_chunk(HTa, 3),
                    lambda: vproj_chunk(l, 2),
                    lambda: qproj_chunk(l, 3),
                    lambda: vproj_chunk(l, 3),
                ]

            # ---------------- layer loop ----------------
            # Software-pipelined across layers: phase A of layer l+1 is
            # split -- chunks 0/1 (transpose+q~+V) emit at the tail of layer
            # l; chunks 2/3 weave into layer l+1's own phase B, so the DVE
            # residual/LN backlog from mlp2 drains under scores/AV cover.
            # layer-0 prologue: HTa is DMA-seeded from transposed h0.
            qproj_chunk(0, 0)
            vproj_chunk(0, 0)
            qproj_chunk(0, 1)
            vproj_chunk(0, 1)

            NL = L * REPEAT
            for li in range(NL):
                l = li % L
                ln = (li + 1) % L  # next layer's weights
                fin = li == NL - 1
                first = li == 0
                # fully software-pipelined body: MLP chunks start as soon as
                # their LN1 lands (under scores/AV cover); next layer's
                # phase-A chunks 0/1 fill the tail, chunks 2/3 weave into
                # the next body's own phase B.
                scores_chunk(0)
                if not first:
                    transpose_chunk(HTa, 2)
                scores_chunk(1)
                qproj_chunk(l, 2)
                if not first:
                    transpose_chunk(HTa, 3)
                apA0 = av_chunk(l, 0)
                qproj_chunk(l, 3)
                vproj_chunk(l, 2)
                scores_chunk(2)
                apA1 = av_chunk(l, 1)
                apA0()
                vproj_chunk(l, 3)
                transpose_chunk(HTm, 0)
                mlp1_chunk(l, 0)
                scores_chunk(3)
                apA2 = av_chunk(l, 2)
                apA1()
                transpose_chunk(HTm, 1)
                mlp1_chunk(l, 1)
                apM0 = mlp2_chunk(l, 0, final=fin)
                apA3 = av_chunk(l, 3)
                apA2()
                transpose_chunk(HTm, 2)
                mlp1_chunk(l, 2)
                apM1 = mlp2_chunk(l, 1, final=fin)
                apM0()
                apA3()
                transpose_chunk(HTm, 3)
                mlp1_chunk(l, 3)
                apM2 = mlp2_chunk(l, 2, final=fin)
                apM1()
                if li < NL - 1:
                    transpose_chunk(HTa, 0)
                    apM3 = mlp2_chunk(l, 3, final=fin)
                    apM2()
                    transpose_chunk(HTa, 1)
                    qproj_chunk(ln, 0)
                    apM3()
                    vproj_chunk(ln, 0)
                    qproj_chunk(ln, 1)
                    vproj_chunk(ln, 1)
                else:
                    apM3 = mlp2_chunk(l, 3, final=fin)
                    apM2()
                    apM3()

    nc.finalize()
    return nc


def _w2ext(w2, bf):
    """bf16 W2 with an extra column of row-sums (of the bf16 weights)."""
    w2b = w2.astype(bf)
    col = w2b.astype(np.float32).sum(-1, keepdims=True).astype(bf)
    return np.ascontiguousarray(np.concatenate([w2b, col], axis=-1))


def kernel(**inputs):
    global LAST_EXEC_NS, LAST_RESULTS
    import ml_dtypes
    from concourse import bass_utils

    bf = ml_dtypes.bfloat16

    x = np.asarray(inputs["x"], dtype=np.float32)
    wpe = np.asarray(inputs["wpe"], dtype=np.float32)
    assert x.shape == (B, N, D), x.shape

    use_b1 = bool(np.any(np.asarray(inputs["mlp_b1"]) != 0))
    use_b2 = bool(np.any(np.asarray(inputs["mlp_b2"]) != 0))
    use_ln1 = not (np.all(np.asarray(inputs["ln1_g"]) == 1)
                   and np.all(np.asarray(inputs["ln1_b"]) == 0))
    use_ln2 = not (np.all(np.asarray(inputs["ln2_g"]) == 1)
                   and np.all(np.asarray(inputs["ln2_b"]) == 0))

    key = (use_b1, use_b2, use_ln1, use_ln2)
    if key not in _CACHE:
        _CACHE[key] = _build_program(*key)
    nc = _CACHE[key]

    h0 = x + wpe[None, :, :]  # positional embedding folded in on host

    Wq = np.asarray(inputs["Wq"], dtype=np.float32)
    Wk = np.asarray(inputs["Wk"], dtype=np.float32)
    M = np.einsum("lde,lfe->ldf", Wq, Wk)  # M[l] = Wq[l] @ Wk[l]^T

    ident = np.eye(P, dtype=np.float32)
    # trimask[k][jj, c] = 1 if c >= 128*k + jj else 0  (keep j <= i in-tile)
    jj = np.arange(P)[:, None]
    cc = np.arange(CW)[None, :]
    tri = np.stack([(cc >= P * k + jj) for k in range(4)]).astype(np.float32)
    pos = np.arange(N, dtype=np.float32).reshape(NT, P).T  # [P, NT]
    invpos = (1.0 / (pos + 1.0)).astype(np.float32)

    shared = {
        "ident": ident.astype(bf),
        "tri": tri.astype(bf),
        "invpos": invpos,
        "m": np.ascontiguousarray(M).astype(bf),
        "wv": np.ascontiguousarray(inputs["Wv"], dtype=np.float32).astype(bf),
        "w1": np.ascontiguousarray(inputs["mlp_W1"],
                                   dtype=np.float32).astype(bf),
        "w2": _w2ext(np.ascontiguousarray(inputs["mlp_W2"],
                                          dtype=np.float32), bf),
    }
    if use_b1:
        shared["b1"] = np.asarray(inputs["mlp_b1"], dtype=np.float32)
    if use_b2:
        shared["b2"] = np.asarray(inputs["mlp_b2"],
                                  dtype=np.float32).astype(bf)
    if use_ln1:
        shared["ln1g"] = np.asarray(inputs["ln1_g"],
                                    dtype=np.float32).astype(bf)
        shared["ln1b"] = np.asarray(inputs["ln1_b"],
                                    dtype=np.float32).astype(bf)
    if use_ln2:
        shared["ln2g"] = np.asarray(inputs["ln2_g"],
                                    dtype=np.float32).astype(bf)
        shared["ln2b"] = np.asarray(inputs["ln2_b"],
                                    dtype=np.float32).astype(bf)

    in_maps = [dict(shared,
                    h0=np.ascontiguousarray(h0[c]).astype(bf),
                    h0t=np.ascontiguousarray(h0[c].T).astype(bf))
               for c in range(B)]
    global _last_in_maps
    _last_in_maps = in_maps

    res = bass_utils.run_bass_kernel_spmd(
        nc, in_maps, core_ids=list(range(B)), trace=PROFILE)
    LAST_EXEC_NS = res.exec_time_ns
    LAST_RESULTS = res
    return np.stack([np.asarray(res.results[c]["out"]).astype(np.float32)
                     for c in range(B)], axis=0)
